# revision 19
# baseline (speedup 1.0000x reference)
"""TRN2 Bass kernel for nn_BlockMoVaE (attention + MoE/VE routing block).

Self-contained: accepts FULL inputs, shards across 8 NeuronCores, returns
FULL output.

Sharding:
  Phase 1 (attention + router logits): token-parallel. Core c handles the
    512-query strip [qoff, qoff+512) of batch b=c//4, qoff=512*(c%4).
    Activations are kept FEATURE-major ([feature, token]) so no on-device
    transposes are needed. K/V are computed for the whole batch on each
    core of the batch group, with key 128-tiles stored in a per-core
    ROTATED slot order (slot s holds absolute key tile (qoff/128+s)%16) so
    the causal boundary lands at static slots 0..3 in every core's
    (shared, SPMD) program; fully-masked future tiles are killed by a
    per-slot additive bias (-3e4) inside the exp activation.
  Phase 2 (expert-parallel sparse MoE): core e computes MLP expert e over
    only the tokens routed to it (host gathers columns, capacity-padded);
    VE (vocab-embedding expert) rows are host-gathered and weighted on
    device per token strip. Host does top-2 routing between launches and
    the final scatter-add/assembly.

Matmuls run as float32r (full PE rate, ~1e-4 rel err); PSUM accumulates
in fp32.
"""
import numpy as np

import concourse.bass as bass
import concourse.bacc as bacc
import concourse.mybir as mybir
import concourse.tile as tile
from concourse.bass_utils import run_bass_kernel_spmd

# ---- problem constants (hardcoded per contest rules) ----
B, T, C = 2, 2048, 1024
NH, NKV, HD = 16, 8, 64
E_MLP, E_VE, TOPK = 8, 2, 2
HID = 2048
VOCAB = 50257
EPS = 1e-6
NCORES = 8
S = 512            # tokens per core strip
NSLOT = T // 128   # 16 key tiles per batch
NG = 4             # kv column groups of 512
NCAP = 1024        # expert token capacity (phase 2)

f32 = mybir.dt.float32
f32r = mybir.dt.float32r
bf16 = mybir.dt.bfloat16
AF = mybir.ActivationFunctionType

_prog_cache = {}



def _register_consts(nc, values):
    for value in values:
        key = (f32, float(value))
        if key not in nc.const_aps.aps:
            t = nc.alloc_sbuf_tensor(f"constap-{value}", [128, 1], f32)
            nc.gpsimd.memset(t.ap(), float(value))
            nc.const_aps.aps[key] = t.ap()
    nc.all_engine_barrier()


# --------------------------------------------------------------------------
# Phase 1 builder: attention + residual + rmsnorm + router logits
# --------------------------------------------------------------------------
def build_phase1(window: int):
    nc = bacc.Bacc("TRN2", target_bir_lowering=False, debug=False,
                   num_devices=NCORES)

    xT_s = nc.dram_tensor("xT_s", [C, S], f32r, kind="ExternalInput").ap()
    xT_b = nc.dram_tensor("xT_b", [C, T], f32r, kind="ExternalInput").ap()
    cosR_s = nc.dram_tensor("cosR_s", [128, S], f32, kind="ExternalInput").ap()
    sinR_s = nc.dram_tensor("sinR_s", [128, S], f32, kind="ExternalInput").ap()
    cosR_b = nc.dram_tensor("cosR_b", [128, T], f32, kind="ExternalInput").ap()
    sinR_b = nc.dram_tensor("sinR_b", [128, T], f32, kind="ExternalInput").ap()
    kbias = nc.dram_tensor("kbias", [128, NSLOT], f32, kind="ExternalInput").ap()
    wqT = nc.dram_tensor("wqT", [C, NH * HD], f32r, kind="ExternalInput").ap()
    wkT = nc.dram_tensor("wkT", [C, NKV * HD], f32r, kind="ExternalInput").ap()
    wvT = nc.dram_tensor("wvT", [C, NKV * HD], f32r, kind="ExternalInput").ap()
    woT = nc.dram_tensor("woT", [C, C], f32r, kind="ExternalInput").ap()
    rwT = nc.dram_tensor("rwT", [C, E_MLP + E_VE], f32, kind="ExternalInput").ap()

    x2_out = nc.dram_tensor("x2_out", [C, S], f32, kind="ExternalOutput").ap()
    xf_out = nc.dram_tensor("xf_out", [C, S], f32r, kind="ExternalOutput").ap()
    logit_out = nc.dram_tensor("logit_out", [E_MLP + E_VE, S], f32,
                               kind="ExternalOutput").ap()

    _register_consts(nc, [EPS])
    from contextlib import ExitStack
    with tile.TileContext(nc) as tc, ExitStack() as est:
        const = est.enter_context(tc.tile_pool(name="const", bufs=1))
        ropes = est.enter_context(tc.tile_pool(name="ropes", bufs=1))
        ropeb = est.enter_context(tc.tile_pool(name="ropeb", bufs=1))
        wstream = est.enter_context(tc.tile_pool(name="wstream", bufs=3))
        wvp = est.enter_context(tc.tile_pool(name="wvp", bufs=1))
        xin = est.enter_context(tc.tile_pool(name="xin", bufs=1))
        kvp = est.enter_context(tc.tile_pool(name="kv", bufs=1))
        qp = est.enter_context(tc.tile_pool(name="qp", bufs=1))
        work = est.enter_context(tc.tile_pool(name="work", bufs=2))
        rows = est.enter_context(tc.tile_pool(name="rows", bufs=2))
        pexp = est.enter_context(tc.tile_pool(name="pexp", bufs=2))
        ypool = est.enter_context(tc.tile_pool(name="ypool", bufs=1))
        x2p = est.enter_context(tc.tile_pool(name="x2p", bufs=2))
        ps_acc = est.enter_context(tc.tile_pool(name="ps_acc", bufs=2, space="PSUM"))
        ps_row = est.enter_context(tc.tile_pool(name="ps_row", bufs=2, space="PSUM"))
        ps_bc = est.enter_context(tc.tile_pool(name="ps_bc", bufs=1, space="PSUM"))
        ps_att = est.enter_context(tc.tile_pool(name="ps_att", bufs=2, space="PSUM"))

        # ---- constants ----
        ones_col = const.tile([128, 1], f32, name="ones_col")
        nc.vector.memset(ones_col[:], 1.0)
        ones_row = const.tile([1, 128], f32, name="ones_row")
        nc.vector.memset(ones_row[:], 1.0)
        onescols = const.tile([128, NKV, 1], f32, name="onescols")
        nc.vector.memset(onescols[:], 1.0)
        onescols_r = const.tile([128, NKV, 1], f32r, name="onescols_r")
        nc.vector.tensor_copy(onescols_r[:], onescols[:])
        kb = const.tile([128, NSLOT], f32, name="kb")
        nc.sync.dma_start(kb[:], kbias[:])

        cs = ropes.tile([128, S], f32, name="cs")
        nc.sync.dma_start(cs[:], cosR_s[:])
        ss = ropes.tile([128, S], f32, name="ss")
        nc.sync.dma_start(ss[:], sinR_s[:])

        rw_t = [const.tile([128, E_MLP + E_VE], f32, tag=f"rw{i}",
                           name=f"rw{i}") for i in range(8)]
        wv_t = [wvp.tile([128, NKV * HD], f32r, tag=f"wv{i}", name=f"wv{i}")
                for i in range(8)]
        for i in range(8):
            nc.sync.dma_start(rw_t[i][:], rwT[bass.ts(i, 128), :])
            nc.sync.dma_start(wv_t[i][:], wvT[bass.ts(i, 128), :])

        # ---- helper: rms broadcast for feature-major tiles ----
        def rms_stats(xtiles, n, nfeat):
            ssq = ps_row.tile([1, n], f32, tag="row", name="ssq")
            for i, xt in enumerate(xtiles):
                sq = work.tile([128, n], f32, tag="sqstat", name="sqstat")
                nc.vector.tensor_mul(sq[:], xt[:], xt[:])
                nc.tensor.matmul(ssq[:], ones_col[:], sq[:],
                                 start=(i == 0), stop=(i == len(xtiles) - 1))
            srow = rows.tile([1, n], f32, tag="srow", name="srow")
            nc.scalar.activation(srow[:], ssq[:], AF.Sqrt,
                                 bias=EPS, scale=1.0 / nfeat)
            rrow = rows.tile([1, n], f32, tag="rrow", name="rrow")
            nc.vector.reciprocal(rrow[:], srow[:])
            bc = ps_bc.tile([128, n], f32, tag="bc", name="bc")
            nc.tensor.matmul(bc[:], ones_row[:], rrow[:], start=True, stop=True)
            bcs = work.tile([128, n], f32, tag="bcstat", name="bcstat", bufs=1)
            nc.scalar.copy(bcs[:], bc[:])
            return bcs

        # ---- helper: rope + per-head rmsnorm on a projection psum ----
        def rope_norm(ps, cos_ap, sin_ap, n, out_tile, col0):
            swp = work.tile([128, n], f32, tag="swp", name="swp")
            for blk in range(4):
                src = (blk // 2) * 64 + (1 - blk % 2) * 32
                nc.scalar.copy(swp[blk * 32:(blk + 1) * 32, :],
                               ps[src:src + 32, :])
            t1 = work.tile([128, n], f32, tag="ropet1", name="ropet1")
            nc.vector.tensor_mul(t1[:], ps[:], cos_ap)
            nc.vector.tensor_mul(swp[:], swp[:], sin_ap)
            nc.vector.tensor_add(swp[:], t1[:], swp[:])   # roped value
            sq = work.tile([128, n], f32, tag="ropet1", name="ropesq")
            nc.vector.tensor_mul(sq[:], swp[:], swp[:])
            for hh in range(2):
                p0 = 64 * hh
                ssqh = ps_row.tile([1, n], f32, tag="row", name="ssqh")
                nc.tensor.matmul(ssqh[:], ones_col[p0:p0 + 64, :],
                                 sq[p0:p0 + 64, :], start=True, stop=True)
                srow = rows.tile([1, n], f32, tag="srow", name="hsrow")
                nc.scalar.activation(srow[:], ssqh[:], AF.Sqrt,
                                     bias=EPS, scale=1.0 / HD)
                rrow = rows.tile([1, n], f32, tag="rrow", name="hrrow")
                nc.vector.reciprocal(rrow[:], srow[:])
                bch = ps_bc.tile([64, n], f32, tag="bc", name="bch")
                nc.tensor.matmul(bch[:], ones_row[:, :64], rrow[:],
                                 start=True, stop=True)
                bcs = work.tile([128, n], f32, tag="hbc", name="hbc")
                nc.scalar.copy(bcs[p0:p0 + 64, :], bch[:])
                nc.vector.tensor_mul(
                    out_tile[p0:p0 + 64, col0:col0 + n],
                    swp[p0:p0 + 64, :], bcs[p0:p0 + 64, :])

        # ================= strip pipeline (Q) =================
        xs_t = [xin.tile([128, S], f32r, tag=f"xi{i}", name=f"xs{i}")
                for i in range(8)]
        for i in range(8):
            nc.sync.dma_start(xs_t[i][:], xT_s[bass.ts(i, 128), :])
        bc_s = rms_stats([t[:].bitcast(f32) for t in xs_t], S, C)
        xn_s = []
        for i in range(8):
            xr = xs_t[i][:]
            nc.vector.tensor_mul(xr, xr.bitcast(f32), bc_s[:])  # in-place norm
            xn_s.append(xr)

        qT = [qp.tile([128, S], f32r, tag=f"qT{i}", name=f"qT{i}")
              for i in range(8)]
        for dq in range(8):
            q_ps = ps_acc.tile([128, S], f32, tag="acc", name="q_ps")
            for ci in range(8):
                wsl = wstream.tile([128, 128], f32r, tag="wq", name="wq_sl")
                nc.sync.dma_start(wsl[:], wqT[bass.ts(ci, 128),
                                              bass.ts(dq, 128)])
                nc.tensor.matmul(q_ps[:], wsl[:], xn_s[ci],
                                 start=(ci == 0), stop=(ci == 7))
            rope_norm(q_ps, cs[:], ss[:], S, qT[dq], 0)

        # ================= batch pipeline (K, V) =================
        kT = [kvp.tile([128, T], f32r, tag=f"kT{i}", name=f"kT{i}")
              for i in range(4)]
        vaug = [kvp.tile([128, NKV * (HD + 1)], f32r, tag=f"va{i}",
                         name=f"va{i}") for i in range(NSLOT)]
        for g in range(NG):
            xb_t = [xin.tile([128, S], f32r, tag=f"xi{i}", name=f"xb{i}")
                    for i in range(8)]
            for i in range(8):
                nc.sync.dma_start(xb_t[i][:], xT_b[bass.ts(i, 128),
                                                   bass.ts(g, S)])
            cbg = ropeb.tile([128, S], f32, tag="cbg", name="cbg")
            nc.sync.dma_start(cbg[:], cosR_b[:, bass.ts(g, S)])
            sbg = ropeb.tile([128, S], f32, tag="sbg", name="sbg")
            nc.sync.dma_start(sbg[:], sinR_b[:, bass.ts(g, S)])
            bc_b = rms_stats([t[:].bitcast(f32) for t in xb_t], S, C)
            xn_b = []
            for i in range(8):
                xr = xb_t[i][:]
                nc.vector.tensor_mul(xr, xr.bitcast(f32), bc_b[:])
                xn_b.append(xr)
            for dk in range(4):
                k_ps = ps_acc.tile([128, S], f32, tag="acc", name="k_ps")
                for ci in range(8):
                    wsl = wstream.tile([128, 128], f32r, tag="wk", name="wk_sl")
                    nc.sync.dma_start(wsl[:], wkT[bass.ts(ci, 128),
                                                  bass.ts(dk, 128)])
                    nc.tensor.matmul(k_ps[:], wsl[:], xn_b[ci],
                                     start=(ci == 0), stop=(ci == 7))
                rope_norm(k_ps, cbg[:], sbg[:], S, kT[dk], g * S)
            for tt in range(4):
                slot = g * 4 + tt
                v_ps = ps_acc.tile([128, NKV * HD], f32, tag="acc", name="v_ps")
                for ci in range(8):
                    nc.tensor.matmul(v_ps[:],
                                     xn_b[ci][:, bass.ts(tt, 128)],
                                     wv_t[ci][:], start=(ci == 0), stop=(ci == 7))
                va = vaug[slot]
                va3 = va[:].rearrange("p (h d) -> p h d", d=HD + 1)
                vp3 = v_ps[:].rearrange("p (h d) -> p h d", d=HD)
                nc.vector.tensor_copy(va3[:, :, 0:HD], vp3[:, :, :])
                nc.vector.tensor_copy(va3[:, :, HD:HD + 1], onescols_r[:])

        # ================= attention =================
        yT = [ypool.tile([128, S], f32r, tag=f"yT{i}", name=f"yT{i}")
              for i in range(8)]
        for h in range(NH):
            kh = h // 2                       # kv head
            dk, kp0 = kh // 2, 64 * (kh % 2)  # kT chunk/partition offset
            # q head layout is host-permuted so its partition base matches
            # the kv head base (matmul requires equal bases)
            dq, qp0 = 2 * (h // 4) + (h % 2), 64 * ((h // 2) % 2)
            assert qp0 == kp0
            yv = ps_att.tile([HD + 1, S], f32, tag="yv", name="yv", bufs=1)
            for s in range(NSLOT):
                s_ps = ps_att.tile([128, S], f32, tag="sps", name="s_ps")
                nc.tensor.matmul(
                    s_ps[:], kT[dk][kp0:kp0 + 64, bass.ts(s, 128)],
                    qT[dq][qp0:qp0 + 64, :], start=True, stop=True)
                pT = pexp.tile([128, S], f32r, tag="pT", name="pT")
                nc.scalar.activation(pT[:], s_ps[:], AF.Exp,
                                     bias=kb[:, s:s + 1], scale=0.125)
                if s < 4:
                    nc.gpsimd.affine_select(
                        pT[:], pT[:], pattern=[[1, S]], base=-128 * s,
                        channel_multiplier=-1,
                        compare_op=mybir.AluOpType.is_ge, fill=0.0)
                    if window < 512 - 128 * s:
                        nc.gpsimd.affine_select(
                            pT[:], pT[:], pattern=[[1, S]],
                            base=-128 * s - window, channel_multiplier=-1,
                            compare_op=mybir.AluOpType.is_le, fill=0.0)
                else:
                    m = NSLOT - s
                    if 128 * m - 127 <= window < 128 * m + 511:
                        nc.gpsimd.affine_select(
                            pT[:], pT[:], pattern=[[1, S]],
                            base=128 * m - window, channel_multiplier=-1,
                            compare_op=mybir.AluOpType.is_le, fill=0.0)
                nc.tensor.matmul(yv[:], vaug[s][:, 65 * kh:65 * kh + 65],
                                 pT[:], start=(s == 0), stop=(s == NSLOT - 1))
            ry = rows.tile([1, S], f32, tag="ry", name="ry")
            nc.vector.reciprocal(ry[:], yv[HD:HD + 1, :])
            bc_y = ps_bc.tile([64, S], f32, tag="bc", name="bc_y")
            nc.tensor.matmul(bc_y[:], ones_row[:, :64], ry[:],
                             start=True, stop=True)
            bcy_s = work.tile([128, S], f32, tag="hbc", name="bcy")
            nc.scalar.copy(bcy_s[qp0:qp0 + 64, :], bc_y[:])
            nc.vector.tensor_mul(yT[dq][qp0:qp0 + 64, :], yv[0:HD, :],
                                 bcy_s[qp0:qp0 + 64, :])

        # ================= wo + residual + xf + router =================
        x2w = []
        for co in range(8):
            at_ps = ps_acc.tile([128, S], f32, tag="acc", name="at_ps")
            for ci in range(8):
                wsl = wstream.tile([128, 128], f32r, tag="wo", name="wo_sl")
                nc.sync.dma_start(wsl[:], woT[bass.ts(ci, 128),
                                              bass.ts(co, 128)])
                nc.tensor.matmul(at_ps[:], wsl[:], yT[ci][:],
                                 start=(ci == 0), stop=(ci == 7))
            xs2 = xin.tile([128, S], f32r, tag=f"xi{co}", name=f"xs2_{co}")
            nc.sync.dma_start(xs2[:], xT_s[bass.ts(co, 128), :])
            x2 = x2p.tile([128, S], f32, tag="x2w", name="x2w")
            nc.vector.tensor_add(x2[:], at_ps[:], xs2[:].bitcast(f32))
            nc.sync.dma_start(x2_out[bass.ts(co, 128), :], x2[:])
            x2w.append(x2)
        # xf stats need all 8 chunks: re-read x2 from DRAM to save SBUF
        x2r = [xin.tile([128, S], f32, tag=f"xi{i}", name=f"x2r{i}")
               for i in range(8)]
        for i in range(8):
            nc.sync.dma_start(x2r[i][:], x2_out[bass.ts(i, 128), :])
        bc_f = rms_stats(x2r, S, C)
        rt_ps = ps_row.tile([E_MLP + E_VE, S], f32, tag="row", name="rt_ps")
        for i in range(8):
            xf = x2p.tile([128, S], f32r, tag="xf", name="xf")
            nc.vector.tensor_mul(xf[:], x2r[i][:], bc_f[:])
            nc.sync.dma_start(xf_out[bass.ts(i, 128), :], xf[:])
            nc.tensor.matmul(rt_ps[:], rw_t[i][:], xf[:].bitcast(f32),
                             start=(i == 0), stop=(i == 7))
        lg = rows.tile([E_MLP + E_VE, S], f32, tag="lg", name="lg", bufs=1)
        nc.scalar.copy(lg[:], rt_ps[:])
        nc.sync.dma_start(logit_out[:], lg[:])

    nc.compile()
    return nc


# --------------------------------------------------------------------------
# Phase 2 builder: sparse expert MLP + VE weighting
# --------------------------------------------------------------------------
def build_phase2(ncap: int):
    nc = bacc.Bacc("TRN2", target_bir_lowering=False, debug=False,
                   num_devices=NCORES)
    NT = ncap // 256

    xfg = nc.dram_tensor("xfg", [C, ncap], f32r, kind="ExternalInput").ap()
    w_upT = nc.dram_tensor("w_upT", [C, HID], f32r, kind="ExternalInput").ap()
    w_downT = nc.dram_tensor("w_downT", [HID, C], f32r,
                             kind="ExternalInput").ap()
    gate = nc.dram_tensor("gate", [1, ncap], f32, kind="ExternalInput").ap()
    ve0 = nc.dram_tensor("ve0", [S, C], f32, kind="ExternalInput").ap()
    ve1 = nc.dram_tensor("ve1", [S, C], f32, kind="ExternalInput").ap()
    ve_g = nc.dram_tensor("ve_g", [128, 8], f32, kind="ExternalInput").ap()

    moe_out = nc.dram_tensor("moe_out", [C, ncap], f32, kind="ExternalOutput").ap()
    ve_out = nc.dram_tensor("ve_out", [S, C], f32, kind="ExternalOutput").ap()

    from contextlib import ExitStack
    with tile.TileContext(nc) as tc, ExitStack() as est:
        const = est.enter_context(tc.tile_pool(name="const", bufs=1))
        wpool = est.enter_context(tc.tile_pool(name="wpool", bufs=1))
        hpool = est.enter_context(tc.tile_pool(name="hpool", bufs=1))
        stream = est.enter_context(tc.tile_pool(name="stream", bufs=2))
        work = est.enter_context(tc.tile_pool(name="work", bufs=2))
        ps_h = est.enter_context(tc.tile_pool(name="ps_h", bufs=2, space="PSUM"))
        ps_o = est.enter_context(tc.tile_pool(name="ps_o", bufs=2, space="PSUM"))
        ps_b = est.enter_context(tc.tile_pool(name="ps_b", bufs=2, space="PSUM"))

        ones_row = const.tile([1, 128], f32)
        nc.vector.memset(ones_row[:], 1.0)

        up_t = [wpool.tile([128, HID], f32r, tag=f"up{i}", name=f"up{i}") for i in range(8)]
        dn_t = [wpool.tile([128, C], f32r, tag=f"dn{i}", name=f"dn{i}") for i in range(16)]
        for i in range(8):
            nc.sync.dma_start(up_t[i][:], w_upT[bass.ts(i, 128), :])
        for i in range(16):
            nc.sync.dma_start(dn_t[i][:], w_downT[bass.ts(i, 128), :])
        veg = const.tile([128, 8], f32)
        nc.sync.dma_start(veg[:], ve_g[:])
        gate_sb = const.tile([1, ncap], f32)
        nc.sync.dma_start(gate_sb[:], gate[:])

        for nt in range(NT):
            csl = bass.ts(nt, 256)
            xf_t = [stream.tile([128, 256], f32r, tag=f"xf{i}", name=f"xf{i}")
                    for i in range(8)]
            for i in range(8):
                nc.sync.dma_start(xf_t[i][:], xfg[bass.ts(i, 128), csl])
            g_ps = ps_b.tile([128, 256], f32)
            nc.tensor.matmul(g_ps[:], ones_row[:], gate_sb[:, csl],
                             start=True, stop=True)
            g_bc = work.tile([128, 256], f32, tag="gbc", name="gbc")
            nc.scalar.copy(g_bc[:], g_ps[:])
            hT = [hpool.tile([128, 256], f32r, tag=f"hT{i}", name=f"hT{i}")
                  for i in range(16)]
            for hc in range(16):
                h_ps = ps_h.tile([128, 256], f32)
                for ci in range(8):
                    nc.tensor.matmul(h_ps[:], up_t[ci][:, bass.ts(hc, 128)],
                                     xf_t[ci][:], start=(ci == 0),
                                     stop=(ci == 7))
                hr = work.tile([128, 256], f32, tag="hrelu", name="hrelu")
                nc.scalar.activation(hr[:], h_ps[:], AF.Relu)
                nc.vector.tensor_mul(hT[hc][:], hr[:], hr[:])
            for co in range(8):
                o_ps = ps_o.tile([128, 256], f32)
                for hc in range(16):
                    nc.tensor.matmul(o_ps[:], dn_t[hc][:, bass.ts(co, 128)],
                                     hT[hc][:], start=(hc == 0),
                                     stop=(hc == 15))
                ot = work.tile([128, 256], f32, tag="ot", name="ot")
                nc.vector.tensor_mul(ot[:], o_ps[:], g_bc[:])
                nc.sync.dma_start(moe_out[bass.ts(co, 128), csl], ot[:])

        # VE weighting for own token strip (token-major)
        for tt in range(4):
            rsl = bass.ts(tt, 128)
            r0 = stream.tile([128, C], f32, tag="ver0", name="ver0")
            r1 = stream.tile([128, C], f32, tag="ver1", name="ver1")
            nc.sync.dma_start(r0[:], ve0[rsl, :])
            nc.sync.dma_start(r1[:], ve1[rsl, :])
            nc.vector.tensor_scalar_mul(r0[:], r0[:], veg[:, 2 * tt:2 * tt + 1])
            nc.vector.tensor_scalar_mul(r1[:], r1[:],
                                        veg[:, 2 * tt + 1:2 * tt + 2])
            nc.vector.tensor_add(r0[:], r0[:], r1[:])
            nc.sync.dma_start(ve_out[rsl, :], r0[:])

    nc.compile()
    return nc


# --------------------------------------------------------------------------
# Host orchestration
# --------------------------------------------------------------------------
def _phase1_inputs(x, cos, sin, window, wq, wk, wv, wo, router_w):
    """Build per-core in_maps for phase 1."""
    cosT = np.ascontiguousarray(cos[0, :, 0, :].T)  # (32, T)
    sinT = np.ascontiguousarray(sin[0, :, 0, :].T)
    cosR = np.tile(cosT, (4, 1)).astype(np.float32)          # (128, T)
    sinR = np.tile(np.vstack([sinT, -sinT]), (2, 1)).astype(np.float32)

    # q-head placement permutation (see attention loop): head h lives at
    # chunk 2*(h//4)+(h%2), partition base 64*((h//2)%2)
    colmap = np.zeros(NH * HD, np.int64)
    for h in range(NH):
        pos = (2 * (h // 4) + (h % 2)) * 128 + 64 * ((h // 2) % 2)
        colmap[pos:pos + HD] = np.arange(h * HD, (h + 1) * HD)
    wqT = np.ascontiguousarray(wq.T[:, colmap])
    wkT = np.ascontiguousarray(wk.T)
    wvT = np.ascontiguousarray(wv.T)
    woT = np.ascontiguousarray(wo.T[colmap, :])
    rwT = np.ascontiguousarray(router_w.T)

    in_maps = []
    perms = []
    for c in range(NCORES):
        b, qi = c // 4, c % 4
        qoff = S * qi
        q128 = qoff // 128
        perm = [(q128 + s) % NSLOT for s in range(NSLOT)]
        perms.append(perm)
        xT = x[b].T  # (C, T)
        xT_rot = np.ascontiguousarray(
            xT.reshape(C, NSLOT, 128)[:, perm, :].reshape(C, T))
        cosR_b = np.ascontiguousarray(
            cosR.reshape(128, NSLOT, 128)[:, perm, :].reshape(128, T))
        sinR_b = np.ascontiguousarray(
            sinR.reshape(128, NSLOT, 128)[:, perm, :].reshape(128, T))
        # per-slot alive bias
        kbias = np.zeros((128, NSLOT), np.float32)
        for s in range(NSLOT):
            kt = perm[s]
            # any (q in [qoff, qoff+511], k in [kt*128, kt*128+127]) with
            # k <= q and q - k <= window?
            dmin = qoff - (kt * 128 + 127)
            dmax = qoff + S - 1 - kt * 128
            alive = (dmax >= 0) and (dmin <= window)
            if not alive:
                kbias[:, s] = -30000.0
        in_maps.append(dict(
            xT_s=np.ascontiguousarray(xT[:, qoff:qoff + S]),
            xT_b=xT_rot,
            cosR_s=np.ascontiguousarray(cosR[:, qoff:qoff + S]),
            sinR_s=np.ascontiguousarray(sinR[:, qoff:qoff + S]),
            cosR_b=cosR_b, sinR_b=sinR_b, kbias=kbias,
            wqT=wqT, wkT=wkT, wvT=wvT, woT=woT, rwT=rwT,
        ))
    return in_maps, perms


def _route(logits, router_bias):
    """Top-2 routing exactly as the reference (on host, f32)."""
    sig = (1.0 / (1.0 + np.exp(-logits.astype(np.float32)))).astype(np.float32)
    sel = sig + router_bias[None, :].astype(np.float32)
    idx = np.argsort(-sel, axis=1, kind="stable")[:, :TOPK]
    tw = np.take_along_axis(sig, idx, axis=1)
    tw = tw / tw.sum(axis=1, keepdims=True)
    N = logits.shape[0]
    sparse_w = np.zeros((N, E_MLP + E_VE), np.float32)
    np.put_along_axis(sparse_w, idx, tw, axis=1)
    return sparse_w


def kernel(**inputs):
    x = np.asarray(inputs["x"], np.float32)
    token_ids = np.asarray(inputs["token_ids"])
    cos = np.asarray(inputs["cos"], np.float32)
    sin = np.asarray(inputs["sin"], np.float32)
    window = int(np.asarray(inputs["window_size"]))
    wq, wk, wv, wo = (np.asarray(inputs[k], np.float32)
                      for k in ("wq", "wk", "wv", "wo"))
    w_up = np.asarray(inputs["w_up"], np.float32)
    w_down = np.asarray(inputs["w_down"], np.float32)
    router_w = np.asarray(inputs["router_w"], np.float32)
    router_bias = np.asarray(inputs["router_bias"], np.float32)
    ve_tables = np.asarray(inputs["ve_tables"], np.float32)

    key1 = ("p1", window)
    if key1 not in _prog_cache:
        _prog_cache[key1] = build_phase1(window)
    nc1 = _prog_cache[key1]

    in_maps, _ = _phase1_inputs(x, cos, sin, window, wq, wk, wv, wo, router_w)
    res1 = run_bass_kernel_spmd(nc1, in_maps, list(range(NCORES))).results

    x2T = np.concatenate([res1[c]["x2_out"] for c in range(NCORES)], axis=1)
    xfT = np.concatenate([res1[c]["xf_out"] for c in range(NCORES)], axis=1)
    logits = np.concatenate([res1[c]["logit_out"].T for c in range(NCORES)],
                            axis=0)  # (N, 10)

    N = B * T
    sparse_w = _route(logits, router_bias)

    # dispatch
    ncap = NCAP
    idx_list, n_list = [], []
    for e in range(E_MLP):
        idx_e = np.nonzero(sparse_w[:, e])[0]
        idx_list.append(idx_e)
        n_list.append(len(idx_e))
    max_n = max(n_list)
    while ncap < max_n:
        ncap *= 2

    key2 = ("p2", ncap)
    if key2 not in _prog_cache:
        _prog_cache[key2] = build_phase2(ncap)
    nc2 = _prog_cache[key2]

    tok = token_ids.reshape(-1)
    in_maps2 = []
    for c in range(NCORES):
        e = c
        idx_e = idx_list[e]
        xfg = np.zeros((C, ncap), np.float32)
        xfg[:, :n_list[e]] = xfT[:, idx_e]
        gate = np.zeros((1, ncap), np.float32)
        gate[0, :n_list[e]] = sparse_w[idx_e, e]
        s0 = c * S
        strip_tok = tok[s0:s0 + S]
        ve0 = np.ascontiguousarray(ve_tables[0][strip_tok])
        ve1 = np.ascontiguousarray(ve_tables[1][strip_tok])
        veg = np.zeros((128, 8), np.float32)
        for tt in range(4):
            for ee in range(E_VE):
                veg[:, 2 * tt + ee] = sparse_w[s0 + tt * 128:s0 + (tt + 1) * 128,
                                               E_MLP + ee]
        in_maps2.append(dict(
            xfg=xfg, w_upT=np.ascontiguousarray(w_up[e].T),
            w_downT=np.ascontiguousarray(w_down[e].T),
            gate=gate, ve0=ve0, ve1=ve1, ve_g=veg,
        ))
    res2 = run_bass_kernel_spmd(nc2, in_maps2, list(range(NCORES))).results

    out = np.ascontiguousarray(x2T.T)  # (N, C)
    for c in range(NCORES):
        out[c * S:(c + 1) * S] += res2[c]["ve_out"]
    for e in range(E_MLP):
        n_e = n_list[e]
        if n_e:
            out[idx_list[e]] += res2[e]["moe_out"][:, :n_e].T
    return out.reshape(B, T, C).astype(np.float32)


# revision 22
# speedup vs baseline: 1.0429x; 1.0429x over previous
"""TRN2 Bass kernel for nn_BlockMoVaE (attention + MoE/VE routing block).

Self-contained: accepts FULL inputs, shards across 8 NeuronCores, returns
FULL output.

Sharding:
  Phase 1 (attention + router logits): token-parallel. Core c handles the
    512-query strip [qoff, qoff+512) of batch b=c//4, qoff=512*(c%4).
    Activations are kept FEATURE-major ([feature, token]) so no on-device
    transposes are needed. K/V are computed for the whole batch on each
    core of the batch group, with key 128-tiles stored in a per-core
    ROTATED slot order (slot s holds absolute key tile (qoff/128+s)%16) so
    the causal boundary lands at static slots 0..3 in every core's
    (shared, SPMD) program; fully-masked future tiles are killed by a
    per-slot additive bias (-3e4) inside the exp activation.
  Phase 2 (expert-parallel sparse MoE): core e computes MLP expert e over
    only the tokens routed to it (host gathers columns, capacity-padded);
    VE (vocab-embedding expert) rows are host-gathered and weighted on
    device per token strip. Host does top-2 routing between launches and
    the final scatter-add/assembly.

Matmuls run as float32r (full PE rate, ~1e-4 rel err); PSUM accumulates
in fp32.
"""
import numpy as np

import concourse.bass as bass
import concourse.bacc as bacc
import concourse.mybir as mybir
import concourse.tile as tile
from concourse.bass_utils import run_bass_kernel_spmd

# ---- problem constants (hardcoded per contest rules) ----
B, T, C = 2, 2048, 1024
NH, NKV, HD = 16, 8, 64
E_MLP, E_VE, TOPK = 8, 2, 2
HID = 2048
VOCAB = 50257
EPS = 1e-6
NCORES = 8
S = 512            # tokens per core strip
NSLOT = T // 128   # 16 key tiles per batch
NG = 4             # kv column groups of 512
NCAP = 1024        # expert token capacity (phase 2)

f32 = mybir.dt.float32
f32r = mybir.dt.float32r
bf16 = mybir.dt.bfloat16
AF = mybir.ActivationFunctionType

_prog_cache = {}



def _register_consts(nc, values):
    for value in values:
        key = (f32, float(value))
        if key not in nc.const_aps.aps:
            t = nc.alloc_sbuf_tensor(f"constap-{value}", [128, 1], f32)
            nc.gpsimd.memset(t.ap(), float(value))
            nc.const_aps.aps[key] = t.ap()
    nc.all_engine_barrier()


# --------------------------------------------------------------------------
# Phase 1 builder: attention + residual + rmsnorm + router logits
# --------------------------------------------------------------------------
def build_phase1(window: int):
    nc = bacc.Bacc("TRN2", target_bir_lowering=False, debug=False,
                   num_devices=NCORES)

    xT_b = nc.dram_tensor("xT_b", [C, T], f32r, kind="ExternalInput").ap()
    xT_s = xT_b[:, 0:S]          # strip == rotated slots 0..3
    cosR_b = nc.dram_tensor("cosR_b", [128, T], f32, kind="ExternalInput").ap()
    sinR_b = nc.dram_tensor("sinR_b", [128, T], f32, kind="ExternalInput").ap()
    cosR_s = cosR_b[:, 0:S]
    sinR_s = sinR_b[:, 0:S]
    kbias = nc.dram_tensor("kbias", [128, NSLOT], f32, kind="ExternalInput").ap()
    wqT = nc.dram_tensor("wqT", [C, NH * HD], f32r, kind="ExternalInput").ap()
    wkT = nc.dram_tensor("wkT", [C, NKV * HD], f32r, kind="ExternalInput").ap()
    wvT = nc.dram_tensor("wvT", [C, NKV * HD], f32r, kind="ExternalInput").ap()
    woT = nc.dram_tensor("woT", [C, C], f32r, kind="ExternalInput").ap()
    rwT = nc.dram_tensor("rwT", [C, E_MLP + E_VE], f32, kind="ExternalInput").ap()

    x2_out = nc.dram_tensor("x2_out", [C, S], f32, kind="ExternalOutput").ap()
    xf_out = nc.dram_tensor("xf_out", [C, S], f32r, kind="ExternalOutput").ap()
    logit_out = nc.dram_tensor("logit_out", [E_MLP + E_VE, S], f32,
                               kind="ExternalOutput").ap()

    _register_consts(nc, [EPS])
    from contextlib import ExitStack
    with tile.TileContext(nc) as tc, ExitStack() as est:
        const = est.enter_context(tc.tile_pool(name="const", bufs=1))
        ropes = est.enter_context(tc.tile_pool(name="ropes", bufs=1))
        ropeb = est.enter_context(tc.tile_pool(name="ropeb", bufs=1))
        wstream = est.enter_context(tc.tile_pool(name="wstream", bufs=3))
        wvp = est.enter_context(tc.tile_pool(name="wvp", bufs=1))
        xin = est.enter_context(tc.tile_pool(name="xin", bufs=1))
        kvp = est.enter_context(tc.tile_pool(name="kv", bufs=1))
        qp = est.enter_context(tc.tile_pool(name="qp", bufs=1))
        work = est.enter_context(tc.tile_pool(name="work", bufs=2))
        rows = est.enter_context(tc.tile_pool(name="rows", bufs=2))
        pexp = est.enter_context(tc.tile_pool(name="pexp", bufs=2))
        ypool = est.enter_context(tc.tile_pool(name="ypool", bufs=1))
        x2p = est.enter_context(tc.tile_pool(name="x2p", bufs=2))
        ps_acc = est.enter_context(tc.tile_pool(name="ps_acc", bufs=2, space="PSUM"))
        ps_row = est.enter_context(tc.tile_pool(name="ps_row", bufs=1, space="PSUM"))
        ps_bc = est.enter_context(tc.tile_pool(name="ps_bc", bufs=1, space="PSUM"))
        ps_att = est.enter_context(tc.tile_pool(name="ps_att", bufs=2, space="PSUM"))

        # ---- constants ----
        ones_col_f = const.tile([128, 1], f32, name="ones_col_f")
        nc.vector.memset(ones_col_f[:], 1.0)
        ones_col = const.tile([128, 1], f32r, name="ones_col")
        nc.scalar.copy(ones_col[:], ones_col_f[:])
        ones_row_f = const.tile([1, 128], f32, name="ones_row_f")
        nc.vector.memset(ones_row_f[:], 1.0)
        ones_row = const.tile([1, 128], f32r, name="ones_row")
        nc.scalar.copy(ones_row[:], ones_row_f[:])
        onescols = const.tile([128, NKV, 1], f32, name="onescols")
        nc.vector.memset(onescols[:], 1.0)
        onescols_r = const.tile([128, NKV, 1], f32r, name="onescols_r")
        nc.vector.tensor_copy(onescols_r[:], onescols[:])
        kb = const.tile([128, NSLOT], f32, name="kb")
        nc.sync.dma_start(kb[:], kbias[:])

        cs = ropes.tile([128, S], f32, name="cs")
        nc.sync.dma_start(cs[:], cosR_s[:])
        ss = ropes.tile([128, S], f32, name="ss")
        nc.sync.dma_start(ss[:], sinR_s[:])

        rw_t = [const.tile([128, E_MLP + E_VE], f32, tag=f"rw{i}",
                           name=f"rw{i}") for i in range(8)]
        wv_t = [wvp.tile([128, NKV * HD], f32r, tag=f"wv{i}", name=f"wv{i}")
                for i in range(8)]
        for i in range(8):
            nc.sync.dma_start(rw_t[i][:], rwT[bass.ts(i, 128), :])
            nc.sync.dma_start(wv_t[i][:], wvT[bass.ts(i, 128), :])

        # ---- helper: rms broadcast for feature-major tiles ----
        def rms_stats(xtiles, n, nfeat):
            ssq = ps_row.tile([1, n], f32, tag="row", name="ssq")
            for i, xt in enumerate(xtiles):
                sq = work.tile([128, n], f32r, tag="sqstat", name="sqstat")
                nc.vector.tensor_mul(sq[:], xt[:], xt[:])
                nc.tensor.matmul(ssq[:], ones_col[:], sq[:],
                                 start=(i == 0), stop=(i == len(xtiles) - 1))
            srow = rows.tile([1, n], f32, tag="srow", name="srow")
            nc.scalar.activation(srow[:], ssq[:], AF.Sqrt,
                                 bias=EPS, scale=1.0 / nfeat)
            rrow = rows.tile([1, n], f32r, tag="rrow", name="rrow")
            with nc.allow_low_precision(reason="f32r rms bcast rows"):
                nc.vector.reciprocal(rrow[:], srow[:])
            bc = ps_bc.tile([128, n], f32, tag="bc", name="bc")
            nc.tensor.matmul(bc[:], ones_row[:], rrow[:], start=True, stop=True)
            bcs = work.tile([128, n], f32, tag="bcstat", name="bcstat", bufs=1)
            nc.scalar.copy(bcs[:], bc[:])
            return bcs

        # ---- helper: rope + per-head rmsnorm on a projection psum ----
        def rope_norm(ps, cos_ap, sin_ap, n, out_tile, col0):
            # swp[blk] = ps[swapped 32-block] * sinR[blk] (psum offset reads)
            swp = work.tile([128, n], f32, tag="swp", name="swp")
            for blk in range(4):
                sb0 = (blk // 2) * 64 + (1 - blk % 2) * 32
                b0 = blk * 32
                nc.vector.tensor_mul(swp[b0:b0 + 32, :], ps[sb0:sb0 + 32, :],
                                     sin_ap[b0:b0 + 32, :])
            t1 = work.tile([128, n], f32, tag="ropet1", name="ropet1")
            nc.vector.tensor_mul(t1[:], ps[:], cos_ap)
            nc.vector.tensor_add(swp[:], t1[:], swp[:])   # roped value
            sq = work.tile([128, n], f32r, tag="ropet1", name="ropesq")
            nc.vector.tensor_mul(sq[:], swp[:], swp[:])
            for hh in range(2):
                p0 = 64 * hh
                ssqh = ps_row.tile([1, n], f32, tag="row", name="ssqh")
                nc.tensor.matmul(ssqh[:], ones_col[p0:p0 + 64, :],
                                 sq[p0:p0 + 64, :], start=True, stop=True)
                srow = rows.tile([1, n], f32, tag="srow", name="hsrow")
                nc.scalar.activation(srow[:], ssqh[:], AF.Sqrt,
                                     bias=EPS, scale=1.0 / HD)
                rrow = rows.tile([1, n], f32r, tag="rrow", name="hrrow")
                with nc.allow_low_precision(reason="f32r rms bcast rows"):
                    nc.vector.reciprocal(rrow[:], srow[:])
                bch = ps_bc.tile([64, n], f32, tag="bc", name="bch")
                nc.tensor.matmul(bch[:], ones_row[:, :64], rrow[:],
                                 start=True, stop=True)
                bcs = work.tile([128, n], f32, tag="hbc", name="hbc")
                nc.scalar.copy(bcs[p0:p0 + 64, :], bch[:])
                nc.vector.tensor_mul(
                    out_tile[p0:p0 + 64, col0:col0 + n],
                    swp[p0:p0 + 64, :], bcs[p0:p0 + 64, :])

        # ================= strip pipeline (Q) =================
        xs_t = [xin.tile([128, S], f32r, tag=f"xi{i}", name=f"xs{i}")
                for i in range(8)]
        for i in range(8):
            nc.sync.dma_start(xs_t[i][:], xT_s[bass.ts(i, 128), :])
        bc_s = rms_stats([t[:].bitcast(f32) for t in xs_t], S, C)
        xn_s = []
        for i in range(8):
            xr = xs_t[i][:]
            nc.vector.tensor_mul(xr, xr.bitcast(f32), bc_s[:])  # in-place norm
            xn_s.append(xr)

        qT = [qp.tile([128, S], f32r, tag=f"qT{i}", name=f"qT{i}")
              for i in range(8)]
        for dq in range(8):
            q_ps = ps_acc.tile([128, S], f32, tag="acc", name="q_ps")
            wsl = wstream.tile([128, C], f32r, tag="wq", name="wq_sl", bufs=2)
            nc.sync.dma_start(
                wsl[:].rearrange("p (a m) -> p a m", m=128),
                wqT[:, bass.ts(dq, 128)].rearrange("(a p) m -> p a m", p=128))
            for ci in range(8):
                nc.tensor.matmul(q_ps[:], wsl[:, bass.ts(ci, 128)], xn_s[ci],
                                 start=(ci == 0), stop=(ci == 7))
            rope_norm(q_ps, cs[:], ss[:], S, qT[dq], 0)

        # ================= batch pipeline (K, V) =================
        kT = [kvp.tile([128, T], f32r, tag=f"kT{i}", name=f"kT{i}")
              for i in range(4)]
        vaug = [kvp.tile([128, NKV * (HD + 1)], f32r, tag=f"va{i}",
                         name=f"va{i}") for i in range(NSLOT)]
        for g in range(NG):
            xb_t = [xin.tile([128, S], f32r, tag=f"xi{i}", name=f"xb{i}")
                    for i in range(8)]
            for i in range(8):
                nc.sync.dma_start(xb_t[i][:], xT_b[bass.ts(i, 128),
                                                   bass.ts(g, S)])
            cbg = ropeb.tile([128, S], f32, tag="cbg", name="cbg")
            nc.sync.dma_start(cbg[:], cosR_b[:, bass.ts(g, S)])
            sbg = ropeb.tile([128, S], f32, tag="sbg", name="sbg")
            nc.sync.dma_start(sbg[:], sinR_b[:, bass.ts(g, S)])
            bc_b = rms_stats([t[:].bitcast(f32) for t in xb_t], S, C)
            xn_b = []
            for i in range(8):
                xr = xb_t[i][:]
                nc.vector.tensor_mul(xr, xr.bitcast(f32), bc_b[:])
                xn_b.append(xr)
            for dk in range(4):
                k_ps = ps_acc.tile([128, S], f32, tag="acc", name="k_ps")
                wsl = wstream.tile([128, C], f32r, tag="wk", name="wk_sl",
                                   bufs=2)
                nc.sync.dma_start(
                    wsl[:].rearrange("p (a m) -> p a m", m=128),
                    wkT[:, bass.ts(dk, 128)].rearrange("(a p) m -> p a m",
                                                       p=128))
                for ci in range(8):
                    nc.tensor.matmul(k_ps[:], wsl[:, bass.ts(ci, 128)],
                                     xn_b[ci], start=(ci == 0), stop=(ci == 7))
                rope_norm(k_ps, cbg[:], sbg[:], S, kT[dk], g * S)
            for tt in range(4):
                slot = g * 4 + tt
                v_ps = ps_acc.tile([128, NKV * HD], f32, tag="acc", name="v_ps")
                for ci in range(8):
                    nc.tensor.matmul(v_ps[:],
                                     xn_b[ci][:, bass.ts(tt, 128)],
                                     wv_t[ci][:], start=(ci == 0), stop=(ci == 7))
                va = vaug[slot]
                va3 = va[:].rearrange("p (h d) -> p h d", d=HD + 1)
                vp3 = v_ps[:].rearrange("p (h d) -> p h d", d=HD)
                nc.vector.tensor_copy(va3[:, :, 0:HD], vp3[:, :, :])
                nc.vector.tensor_copy(va3[:, :, HD:HD + 1], onescols_r[:])

        # ================= attention =================
        yT = [ypool.tile([128, S], f32r, tag=f"yT{i}", name=f"yT{i}")
              for i in range(8)]
        for h in range(NH):
            kh = h // 2                       # kv head
            dk, kp0 = kh // 2, 64 * (kh % 2)  # kT chunk/partition offset
            # q head layout is host-permuted so its partition base matches
            # the kv head base (matmul requires equal bases)
            dq, qp0 = 2 * (h // 4) + (h % 2), 64 * ((h // 2) % 2)
            assert qp0 == kp0
            yv = ps_att.tile([HD + 1, S], f32, tag="yv", name="yv")
            for s in range(NSLOT):
                s_ps = ps_att.tile([128, S], f32, tag="sps", name="s_ps")
                nc.tensor.matmul(
                    s_ps[:], kT[dk][kp0:kp0 + 64, bass.ts(s, 128)],
                    qT[dq][qp0:qp0 + 64, :], start=True, stop=True)
                pT = pexp.tile([128, S], f32r, tag="pT", name="pT")
                nc.scalar.activation(pT[:], s_ps[:], AF.Exp,
                                     bias=kb[:, s:s + 1], scale=0.125)
                if s < 4:
                    nc.gpsimd.affine_select(
                        pT[:], pT[:], pattern=[[1, S]], base=-128 * s,
                        channel_multiplier=-1,
                        compare_op=mybir.AluOpType.is_ge, fill=0.0)
                    if window < 512 - 128 * s:
                        nc.gpsimd.affine_select(
                            pT[:], pT[:], pattern=[[1, S]],
                            base=-128 * s - window, channel_multiplier=-1,
                            compare_op=mybir.AluOpType.is_le, fill=0.0)
                else:
                    m = NSLOT - s
                    if 128 * m - 127 <= window < 128 * m + 511:
                        nc.gpsimd.affine_select(
                            pT[:], pT[:], pattern=[[1, S]],
                            base=128 * m - window, channel_multiplier=-1,
                            compare_op=mybir.AluOpType.is_le, fill=0.0)
                nc.tensor.matmul(yv[:], vaug[s][:, 65 * kh:65 * kh + 65],
                                 pT[:], start=(s == 0), stop=(s == NSLOT - 1))
            ry = rows.tile([1, S], f32r, tag="ry", name="ry")
            with nc.allow_low_precision(reason="f32r softmax denom row"):
                nc.vector.reciprocal(ry[:], yv[HD:HD + 1, :])
            bc_y = ps_bc.tile([64, S], f32, tag="bc", name="bc_y")
            nc.tensor.matmul(bc_y[:], ones_row[:, :64], ry[:],
                             start=True, stop=True)
            bcy_s = work.tile([128, S], f32, tag="hbc", name="bcy")
            nc.scalar.copy(bcy_s[qp0:qp0 + 64, :], bc_y[:])
            nc.vector.tensor_mul(yT[dq][qp0:qp0 + 64, :], yv[0:HD, :],
                                 bcy_s[qp0:qp0 + 64, :])

        # ================= wo + residual + xf + router =================
        x2w = []
        for co in range(8):
            at_ps = ps_acc.tile([128, S], f32, tag="acc", name="at_ps")
            wsl = wstream.tile([128, C], f32r, tag="wo", name="wo_sl", bufs=2)
            nc.sync.dma_start(
                wsl[:].rearrange("p (a m) -> p a m", m=128),
                woT[:, bass.ts(co, 128)].rearrange("(a p) m -> p a m", p=128))
            for ci in range(8):
                nc.tensor.matmul(at_ps[:], wsl[:, bass.ts(ci, 128)],
                                 yT[ci][:], start=(ci == 0), stop=(ci == 7))
            xs2 = xin.tile([128, S], f32r, tag=f"xi{co}", name=f"xs2_{co}")
            nc.sync.dma_start(xs2[:], xT_s[bass.ts(co, 128), :])
            x2 = x2p.tile([128, S], f32, tag="x2w", name="x2w")
            nc.vector.tensor_add(x2[:], at_ps[:], xs2[:].bitcast(f32))
            nc.sync.dma_start(x2_out[bass.ts(co, 128), :], x2[:])
            x2w.append(x2)
        # xf stats need all 8 chunks: re-read x2 from DRAM to save SBUF
        x2r = [xin.tile([128, S], f32, tag=f"xi{i}", name=f"x2r{i}")
               for i in range(8)]
        for i in range(8):
            nc.sync.dma_start(x2r[i][:], x2_out[bass.ts(i, 128), :])
        bc_f = rms_stats(x2r, S, C)
        rt_ps = ps_row.tile([E_MLP + E_VE, S], f32, tag="row", name="rt_ps")
        for i in range(8):
            xf = x2p.tile([128, S], f32r, tag="xf", name="xf")
            nc.vector.tensor_mul(xf[:], x2r[i][:], bc_f[:])
            nc.sync.dma_start(xf_out[bass.ts(i, 128), :], xf[:])
            nc.tensor.matmul(rt_ps[:], rw_t[i][:], xf[:].bitcast(f32),
                             start=(i == 0), stop=(i == 7))
        lg = rows.tile([E_MLP + E_VE, S], f32, tag="lg", name="lg", bufs=1)
        nc.scalar.copy(lg[:], rt_ps[:])
        nc.sync.dma_start(logit_out[:], lg[:])

    nc.compile()
    return nc


# --------------------------------------------------------------------------
# Phase 2 builder: sparse expert MLP + VE weighting
# --------------------------------------------------------------------------
def build_phase2(ncap: int):
    nc = bacc.Bacc("TRN2", target_bir_lowering=False, debug=False,
                   num_devices=NCORES)
    NT = ncap // 256

    xfg = nc.dram_tensor("xfg", [C, ncap], f32r, kind="ExternalInput").ap()
    w_upT = nc.dram_tensor("w_upT", [C, HID], f32r, kind="ExternalInput").ap()
    w_downT = nc.dram_tensor("w_downT", [HID, C], f32r,
                             kind="ExternalInput").ap()
    gate = nc.dram_tensor("gate", [1, ncap], f32, kind="ExternalInput").ap()
    ve0 = nc.dram_tensor("ve0", [S, C], f32, kind="ExternalInput").ap()
    ve1 = nc.dram_tensor("ve1", [S, C], f32, kind="ExternalInput").ap()
    ve_g = nc.dram_tensor("ve_g", [128, 8], f32, kind="ExternalInput").ap()

    moe_out = nc.dram_tensor("moe_out", [C, ncap], f32, kind="ExternalOutput").ap()
    ve_out = nc.dram_tensor("ve_out", [S, C], f32, kind="ExternalOutput").ap()

    from contextlib import ExitStack
    with tile.TileContext(nc) as tc, ExitStack() as est:
        const = est.enter_context(tc.tile_pool(name="const", bufs=1))
        wpool = est.enter_context(tc.tile_pool(name="wpool", bufs=1))
        hpool = est.enter_context(tc.tile_pool(name="hpool", bufs=1))
        stream = est.enter_context(tc.tile_pool(name="stream", bufs=2))
        work = est.enter_context(tc.tile_pool(name="work", bufs=2))
        ps_h = est.enter_context(tc.tile_pool(name="ps_h", bufs=2, space="PSUM"))
        ps_o = est.enter_context(tc.tile_pool(name="ps_o", bufs=2, space="PSUM"))
        ps_b = est.enter_context(tc.tile_pool(name="ps_b", bufs=2, space="PSUM"))

        ones_row = const.tile([1, 128], f32)
        nc.vector.memset(ones_row[:], 1.0)

        up_t = [wpool.tile([128, HID], f32r, tag=f"up{i}", name=f"up{i}") for i in range(8)]
        dn_t = [wpool.tile([128, C], f32r, tag=f"dn{i}", name=f"dn{i}") for i in range(16)]
        for i in range(8):
            nc.sync.dma_start(up_t[i][:], w_upT[bass.ts(i, 128), :])
        for i in range(16):
            nc.sync.dma_start(dn_t[i][:], w_downT[bass.ts(i, 128), :])
        veg = const.tile([128, 8], f32)
        nc.sync.dma_start(veg[:], ve_g[:])
        gate_sb = const.tile([1, ncap], f32)
        nc.sync.dma_start(gate_sb[:], gate[:])

        for nt in range(NT):
            csl = bass.ts(nt, 256)
            xf_t = [stream.tile([128, 256], f32r, tag=f"xf{i}", name=f"xf{i}")
                    for i in range(8)]
            for i in range(8):
                nc.sync.dma_start(xf_t[i][:], xfg[bass.ts(i, 128), csl])
            g_ps = ps_b.tile([128, 256], f32)
            nc.tensor.matmul(g_ps[:], ones_row[:], gate_sb[:, csl],
                             start=True, stop=True)
            g_bc = work.tile([128, 256], f32, tag="gbc", name="gbc")
            nc.scalar.copy(g_bc[:], g_ps[:])
            hT = [hpool.tile([128, 256], f32r, tag=f"hT{i}", name=f"hT{i}")
                  for i in range(16)]
            for hc in range(16):
                h_ps = ps_h.tile([128, 256], f32)
                for ci in range(8):
                    nc.tensor.matmul(h_ps[:], up_t[ci][:, bass.ts(hc, 128)],
                                     xf_t[ci][:], start=(ci == 0),
                                     stop=(ci == 7))
                hr = work.tile([128, 256], f32, tag="hrelu", name="hrelu")
                nc.scalar.activation(hr[:], h_ps[:], AF.Relu)
                nc.vector.tensor_mul(hT[hc][:], hr[:], hr[:])
            for co in range(8):
                o_ps = ps_o.tile([128, 256], f32)
                for hc in range(16):
                    nc.tensor.matmul(o_ps[:], dn_t[hc][:, bass.ts(co, 128)],
                                     hT[hc][:], start=(hc == 0),
                                     stop=(hc == 15))
                ot = work.tile([128, 256], f32, tag="ot", name="ot")
                nc.vector.tensor_mul(ot[:], o_ps[:], g_bc[:])
                nc.sync.dma_start(moe_out[bass.ts(co, 128), csl], ot[:])

        # VE weighting for own token strip (token-major)
        for tt in range(4):
            rsl = bass.ts(tt, 128)
            r0 = stream.tile([128, C], f32, tag="ver0", name="ver0")
            r1 = stream.tile([128, C], f32, tag="ver1", name="ver1")
            nc.sync.dma_start(r0[:], ve0[rsl, :])
            nc.sync.dma_start(r1[:], ve1[rsl, :])
            nc.vector.tensor_scalar_mul(r0[:], r0[:], veg[:, 2 * tt:2 * tt + 1])
            nc.vector.tensor_scalar_mul(r1[:], r1[:],
                                        veg[:, 2 * tt + 1:2 * tt + 2])
            nc.vector.tensor_add(r0[:], r0[:], r1[:])
            nc.sync.dma_start(ve_out[rsl, :], r0[:])

    nc.compile()
    return nc


# --------------------------------------------------------------------------
# Host orchestration
# --------------------------------------------------------------------------
def _phase1_inputs(x, cos, sin, window, wq, wk, wv, wo, router_w):
    """Build per-core in_maps for phase 1."""
    cosT = np.ascontiguousarray(cos[0, :, 0, :].T)  # (32, T)
    sinT = np.ascontiguousarray(sin[0, :, 0, :].T)
    cosR = np.tile(cosT, (4, 1)).astype(np.float32)          # (128, T)
    sinR = np.tile(np.vstack([sinT, -sinT]), (2, 1)).astype(np.float32)

    # q-head placement permutation (see attention loop): head h lives at
    # chunk 2*(h//4)+(h%2), partition base 64*((h//2)%2)
    colmap = np.zeros(NH * HD, np.int64)
    for h in range(NH):
        pos = (2 * (h // 4) + (h % 2)) * 128 + 64 * ((h // 2) % 2)
        colmap[pos:pos + HD] = np.arange(h * HD, (h + 1) * HD)
    wqT = np.ascontiguousarray(wq.T[:, colmap])
    wkT = np.ascontiguousarray(wk.T)
    wvT = np.ascontiguousarray(wv.T)
    woT = np.ascontiguousarray(wo.T[colmap, :])
    rwT = np.ascontiguousarray(router_w.T)

    in_maps = []
    perms = []
    for c in range(NCORES):
        b, qi = c // 4, c % 4
        qoff = S * qi
        q128 = qoff // 128
        perm = [(q128 + s) % NSLOT for s in range(NSLOT)]
        perms.append(perm)
        xT = x[b].T  # (C, T)
        xT_rot = np.ascontiguousarray(
            xT.reshape(C, NSLOT, 128)[:, perm, :].reshape(C, T))
        cosR_b = np.ascontiguousarray(
            cosR.reshape(128, NSLOT, 128)[:, perm, :].reshape(128, T))
        sinR_b = np.ascontiguousarray(
            sinR.reshape(128, NSLOT, 128)[:, perm, :].reshape(128, T))
        # per-slot alive bias
        kbias = np.zeros((128, NSLOT), np.float32)
        for s in range(NSLOT):
            kt = perm[s]
            # any (q in [qoff, qoff+511], k in [kt*128, kt*128+127]) with
            # k <= q and q - k <= window?
            dmin = qoff - (kt * 128 + 127)
            dmax = qoff + S - 1 - kt * 128
            alive = (dmax >= 0) and (dmin <= window)
            if not alive:
                kbias[:, s] = -30000.0
        in_maps.append(dict(
            xT_b=xT_rot,
            cosR_b=cosR_b, sinR_b=sinR_b, kbias=kbias,
            wqT=wqT, wkT=wkT, wvT=wvT, woT=woT, rwT=rwT,
        ))
    return in_maps, perms


def _route(logits, router_bias):
    """Top-2 routing exactly as the reference (on host, f32)."""
    sig = (1.0 / (1.0 + np.exp(-logits.astype(np.float32)))).astype(np.float32)
    sel = sig + router_bias[None, :].astype(np.float32)
    idx = np.argsort(-sel, axis=1, kind="stable")[:, :TOPK]
    tw = np.take_along_axis(sig, idx, axis=1)
    tw = tw / tw.sum(axis=1, keepdims=True)
    N = logits.shape[0]
    sparse_w = np.zeros((N, E_MLP + E_VE), np.float32)
    np.put_along_axis(sparse_w, idx, tw, axis=1)
    return sparse_w


def kernel(**inputs):
    x = np.asarray(inputs["x"], np.float32)
    token_ids = np.asarray(inputs["token_ids"])
    cos = np.asarray(inputs["cos"], np.float32)
    sin = np.asarray(inputs["sin"], np.float32)
    window = int(np.asarray(inputs["window_size"]))
    wq, wk, wv, wo = (np.asarray(inputs[k], np.float32)
                      for k in ("wq", "wk", "wv", "wo"))
    w_up = np.asarray(inputs["w_up"], np.float32)
    w_down = np.asarray(inputs["w_down"], np.float32)
    router_w = np.asarray(inputs["router_w"], np.float32)
    router_bias = np.asarray(inputs["router_bias"], np.float32)
    ve_tables = np.asarray(inputs["ve_tables"], np.float32)

    key1 = ("p1", window)
    if key1 not in _prog_cache:
        _prog_cache[key1] = build_phase1(window)
    nc1 = _prog_cache[key1]

    in_maps, _ = _phase1_inputs(x, cos, sin, window, wq, wk, wv, wo, router_w)
    res1 = run_bass_kernel_spmd(nc1, in_maps, list(range(NCORES))).results

    x2T = np.concatenate([res1[c]["x2_out"] for c in range(NCORES)], axis=1)
    xfT = np.concatenate([res1[c]["xf_out"] for c in range(NCORES)], axis=1)
    logits = np.concatenate([res1[c]["logit_out"].T for c in range(NCORES)],
                            axis=0)  # (N, 10)

    N = B * T
    sparse_w = _route(logits, router_bias)

    # dispatch
    ncap = NCAP
    idx_list, n_list = [], []
    for e in range(E_MLP):
        idx_e = np.nonzero(sparse_w[:, e])[0]
        idx_list.append(idx_e)
        n_list.append(len(idx_e))
    max_n = max(n_list)
    while ncap < max_n:
        ncap *= 2

    key2 = ("p2", ncap)
    if key2 not in _prog_cache:
        _prog_cache[key2] = build_phase2(ncap)
    nc2 = _prog_cache[key2]

    tok = token_ids.reshape(-1)
    in_maps2 = []
    for c in range(NCORES):
        e = c
        idx_e = idx_list[e]
        xfg = np.zeros((C, ncap), np.float32)
        xfg[:, :n_list[e]] = xfT[:, idx_e]
        gate = np.zeros((1, ncap), np.float32)
        gate[0, :n_list[e]] = sparse_w[idx_e, e]
        s0 = c * S
        strip_tok = tok[s0:s0 + S]
        ve0 = np.ascontiguousarray(ve_tables[0][strip_tok])
        ve1 = np.ascontiguousarray(ve_tables[1][strip_tok])
        veg = np.zeros((128, 8), np.float32)
        for tt in range(4):
            for ee in range(E_VE):
                veg[:, 2 * tt + ee] = sparse_w[s0 + tt * 128:s0 + (tt + 1) * 128,
                                               E_MLP + ee]
        in_maps2.append(dict(
            xfg=xfg, w_upT=np.ascontiguousarray(w_up[e].T),
            w_downT=np.ascontiguousarray(w_down[e].T),
            gate=gate, ve0=ve0, ve1=ve1, ve_g=veg,
        ))
    res2 = run_bass_kernel_spmd(nc2, in_maps2, list(range(NCORES))).results

    out = np.ascontiguousarray(x2T.T)  # (N, C)
    for c in range(NCORES):
        out[c * S:(c + 1) * S] += res2[c]["ve_out"]
    for e in range(E_MLP):
        n_e = n_list[e]
        if n_e:
            out[idx_list[e]] += res2[e]["moe_out"][:, :n_e].T
    return out.reshape(B, T, C).astype(np.float32)


# revision 26
# speedup vs baseline: 13019.6069x; 12483.9224x over previous
"""TRN2 Bass kernel for nn_BlockMoVaE (attention + MoE/VE routing block).

Self-contained: accepts FULL inputs, shards across 8 NeuronCores, returns
FULL output.

Sharding:
  Phase 1 (attention + router logits): token-parallel. Core c handles the
    512-query strip [qoff, qoff+512) of batch b=c//4, qoff=512*(c%4).
    Activations are kept FEATURE-major ([feature, token]) so no on-device
    transposes are needed. K/V are computed for the whole batch on each
    core of the batch group, with key 128-tiles stored in a per-core
    ROTATED slot order (slot s holds absolute key tile (qoff/128+s)%16) so
    the causal boundary lands at static slots 0..3 in every core's
    (shared, SPMD) program; fully-masked future tiles are killed by a
    per-slot additive bias (-3e4) inside the exp activation.
  Phase 2 (expert-parallel sparse MoE): core e computes MLP expert e over
    only the tokens routed to it (host gathers columns, capacity-padded);
    VE (vocab-embedding expert) rows are host-gathered and weighted on
    device per token strip. Host does top-2 routing between launches and
    the final scatter-add/assembly.

Matmuls run as float32r (full PE rate, ~1e-4 rel err); PSUM accumulates
in fp32.
"""
import numpy as np

import concourse.bass as bass
import concourse.bacc as bacc
import concourse.mybir as mybir
import concourse.tile as tile
from concourse.bass_utils import run_bass_kernel_spmd

# ---- problem constants (hardcoded per contest rules) ----
B, T, C = 2, 2048, 1024
NH, NKV, HD = 16, 8, 64
E_MLP, E_VE, TOPK = 8, 2, 2
HID = 2048
VOCAB = 50257
EPS = 1e-6
NCORES = 8
S = 512            # tokens per core strip
NSLOT = T // 128   # 16 key tiles per batch
NG = 4             # kv column groups of 512
NCAP = 1024        # expert token capacity (phase 2)

f32 = mybir.dt.float32
f32r = mybir.dt.float32r
bf16 = mybir.dt.bfloat16
AF = mybir.ActivationFunctionType

_prog_cache = {}



def _register_consts(nc, values):
    for value in values:
        key = (f32, float(value))
        if key not in nc.const_aps.aps:
            t = nc.alloc_sbuf_tensor(f"constap-{value}", [128, 1], f32)
            nc.gpsimd.memset(t.ap(), float(value))
            nc.const_aps.aps[key] = t.ap()
    nc.all_engine_barrier()


# --------------------------------------------------------------------------
# Phase 1 builder: attention + residual + rmsnorm + router logits
# --------------------------------------------------------------------------
def build_phase1(window: int):
    nc = bacc.Bacc("TRN2", target_bir_lowering=False, debug=False,
                   num_devices=NCORES)

    xT_b = nc.dram_tensor("xT_b", [C, T], f32r, kind="ExternalInput").ap()
    xT_s = xT_b[:, 0:S]          # strip == rotated slots 0..3
    cosR_b = nc.dram_tensor("cosR_b", [128, T], f32, kind="ExternalInput").ap()
    sinR_b = nc.dram_tensor("sinR_b", [128, T], f32, kind="ExternalInput").ap()
    cosR_s = cosR_b[:, 0:S]
    sinR_s = sinR_b[:, 0:S]
    kbias = nc.dram_tensor("kbias", [128, NSLOT], f32, kind="ExternalInput").ap()
    wqT = nc.dram_tensor("wqT", [C, NH * HD], f32r, kind="ExternalInput").ap()
    wkT = nc.dram_tensor("wkT", [C, NKV * HD], f32r, kind="ExternalInput").ap()
    wvT = nc.dram_tensor("wvT", [C, NKV * HD], f32r, kind="ExternalInput").ap()
    woT = nc.dram_tensor("woT", [C, C], f32r, kind="ExternalInput").ap()
    rwT = nc.dram_tensor("rwT", [C, E_MLP + E_VE], f32, kind="ExternalInput").ap()

    x2_out = nc.dram_tensor("x2_out", [C, S], f32, kind="ExternalOutput").ap()
    xf_out = nc.dram_tensor("xf_out", [C, S], f32r, kind="ExternalOutput").ap()
    logit_out = nc.dram_tensor("logit_out", [E_MLP + E_VE, S], f32,
                               kind="ExternalOutput").ap()

    _register_consts(nc, [EPS])
    from contextlib import ExitStack
    with tile.TileContext(nc) as tc, ExitStack() as est:
        const = est.enter_context(tc.tile_pool(name="const", bufs=1))
        ropes = est.enter_context(tc.tile_pool(name="ropes", bufs=1))
        ropeb = est.enter_context(tc.tile_pool(name="ropeb", bufs=1))
        wstream = est.enter_context(tc.tile_pool(name="wstream", bufs=3))
        wvp = est.enter_context(tc.tile_pool(name="wvp", bufs=1))
        xin = est.enter_context(tc.tile_pool(name="xin", bufs=1))
        kvp = est.enter_context(tc.tile_pool(name="kv", bufs=1))
        qp = est.enter_context(tc.tile_pool(name="qp", bufs=1))
        work = est.enter_context(tc.tile_pool(name="work", bufs=2))
        rows = est.enter_context(tc.tile_pool(name="rows", bufs=2))
        pexp = est.enter_context(tc.tile_pool(name="pexp", bufs=2))
        ypool = est.enter_context(tc.tile_pool(name="ypool", bufs=1))
        x2p = est.enter_context(tc.tile_pool(name="x2p", bufs=2))
        ps_acc = est.enter_context(tc.tile_pool(name="ps_acc", bufs=2, space="PSUM"))
        ps_row = est.enter_context(tc.tile_pool(name="ps_row", bufs=1, space="PSUM"))
        ps_bc = est.enter_context(tc.tile_pool(name="ps_bc", bufs=1, space="PSUM"))
        ps_att = est.enter_context(tc.tile_pool(name="ps_att", bufs=2, space="PSUM"))

        # ---- constants ----
        ones_col_f = const.tile([128, 1], f32, name="ones_col_f")
        nc.vector.memset(ones_col_f[:], 1.0)
        ones_col = const.tile([128, 1], f32r, name="ones_col")
        nc.scalar.copy(ones_col[:], ones_col_f[:])
        ones_row_f = const.tile([1, 128], f32, name="ones_row_f")
        nc.vector.memset(ones_row_f[:], 1.0)
        ones_row = const.tile([1, 128], f32r, name="ones_row")
        nc.scalar.copy(ones_row[:], ones_row_f[:])
        onescols = const.tile([128, NKV, 1], f32, name="onescols")
        nc.vector.memset(onescols[:], 1.0)
        onescols_r = const.tile([128, NKV, 1], f32r, name="onescols_r")
        nc.vector.tensor_copy(onescols_r[:], onescols[:])
        kb = const.tile([128, NSLOT], f32, name="kb")
        nc.sync.dma_start(kb[:], kbias[:])

        cs = ropes.tile([128, S], f32, name="cs")
        nc.sync.dma_start(cs[:], cosR_s[:])
        ss = ropes.tile([128, S], f32, name="ss")
        nc.sync.dma_start(ss[:], sinR_s[:])

        rw_t = [const.tile([128, E_MLP + E_VE], f32, tag=f"rw{i}",
                           name=f"rw{i}") for i in range(8)]
        wv_t = [wvp.tile([128, NKV * HD], f32r, tag=f"wv{i}", name=f"wv{i}")
                for i in range(8)]
        for i in range(8):
            nc.sync.dma_start(rw_t[i][:], rwT[bass.ts(i, 128), :])
            nc.sync.dma_start(wv_t[i][:], wvT[bass.ts(i, 128), :])

        # ---- helper: rms broadcast for feature-major tiles ----
        def rms_stats(xtiles, n, nfeat):
            ssq = ps_row.tile([1, n], f32, tag="row", name="ssq")
            for i, xt in enumerate(xtiles):
                sq = work.tile([128, n], f32r, tag="sqstat", name="sqstat")
                nc.vector.tensor_mul(sq[:], xt[:], xt[:])
                nc.tensor.matmul(ssq[:], ones_col[:], sq[:],
                                 start=(i == 0), stop=(i == len(xtiles) - 1))
            srow = rows.tile([1, n], f32, tag="srow", name="srow")
            nc.scalar.activation(srow[:], ssq[:], AF.Sqrt,
                                 bias=EPS, scale=1.0 / nfeat)
            rrow = rows.tile([1, n], f32r, tag="rrow", name="rrow")
            with nc.allow_low_precision(reason="f32r rms bcast rows"):
                nc.vector.reciprocal(rrow[:], srow[:])
            bc = ps_bc.tile([128, n], f32, tag="bc", name="bc")
            nc.tensor.matmul(bc[:], ones_row[:], rrow[:], start=True, stop=True)
            bcs = work.tile([128, n], f32, tag="bcstat", name="bcstat", bufs=1)
            nc.scalar.copy(bcs[:], bc[:])
            return bcs

        # ---- helper: rope + per-head rmsnorm on a projection psum ----
        def rope_norm(ps, cos_ap, sin_ap, n, out_tile, col0):
            # swp[blk] = ps[swapped 32-block] * sinR[blk] (psum offset reads)
            swp = work.tile([128, n], f32, tag="swp", name="swp")
            for blk in range(4):
                sb0 = (blk // 2) * 64 + (1 - blk % 2) * 32
                b0 = blk * 32
                nc.vector.tensor_mul(swp[b0:b0 + 32, :], ps[sb0:sb0 + 32, :],
                                     sin_ap[b0:b0 + 32, :])
            t1 = work.tile([128, n], f32, tag="ropet1", name="ropet1")
            nc.vector.tensor_mul(t1[:], ps[:], cos_ap)
            nc.vector.tensor_add(swp[:], t1[:], swp[:])   # roped value
            sq = work.tile([128, n], f32r, tag="ropet1", name="ropesq")
            nc.vector.tensor_mul(sq[:], swp[:], swp[:])
            for hh in range(2):
                p0 = 64 * hh
                ssqh = ps_row.tile([1, n], f32, tag="row", name="ssqh")
                nc.tensor.matmul(ssqh[:], ones_col[p0:p0 + 64, :],
                                 sq[p0:p0 + 64, :], start=True, stop=True)
                srow = rows.tile([1, n], f32, tag="srow", name="hsrow")
                nc.scalar.activation(srow[:], ssqh[:], AF.Sqrt,
                                     bias=EPS, scale=1.0 / HD)
                rrow = rows.tile([1, n], f32r, tag="rrow", name="hrrow")
                with nc.allow_low_precision(reason="f32r rms bcast rows"):
                    nc.vector.reciprocal(rrow[:], srow[:])
                bch = ps_bc.tile([64, n], f32, tag="bc", name="bch")
                nc.tensor.matmul(bch[:], ones_row[:, :64], rrow[:],
                                 start=True, stop=True)
                bcs = work.tile([128, n], f32, tag="hbc", name="hbc")
                nc.scalar.copy(bcs[p0:p0 + 64, :], bch[:])
                nc.vector.tensor_mul(
                    out_tile[p0:p0 + 64, col0:col0 + n],
                    swp[p0:p0 + 64, :], bcs[p0:p0 + 64, :])

        # ================= strip pipeline (Q) =================
        xs_t = [xin.tile([128, S], f32r, tag=f"xi{i}", name=f"xs{i}")
                for i in range(8)]
        for i in range(8):
            nc.sync.dma_start(xs_t[i][:], xT_s[bass.ts(i, 128), :])
        bc_s = rms_stats([t[:].bitcast(f32) for t in xs_t], S, C)
        xn_s = []
        for i in range(8):
            xr = xs_t[i][:]
            nc.vector.tensor_mul(xr, xr.bitcast(f32), bc_s[:])  # in-place norm
            xn_s.append(xr)

        qT = [qp.tile([128, S], f32r, tag=f"qT{i}", name=f"qT{i}")
              for i in range(8)]
        for dq in range(8):
            q_ps = ps_acc.tile([128, S], f32, tag="acc", name="q_ps")
            wsl = wstream.tile([128, C], f32r, tag="wq", name="wq_sl", bufs=2)
            nc.sync.dma_start(
                wsl[:].rearrange("p (a m) -> p a m", m=128),
                wqT[:, bass.ts(dq, 128)].rearrange("(a p) m -> p a m", p=128))
            for ci in range(8):
                nc.tensor.matmul(q_ps[:], wsl[:, bass.ts(ci, 128)], xn_s[ci],
                                 start=(ci == 0), stop=(ci == 7))
            rope_norm(q_ps, cs[:], ss[:], S, qT[dq], 0)

        # ================= batch pipeline (K, V) =================
        kT = [kvp.tile([128, T], f32r, tag=f"kT{i}", name=f"kT{i}")
              for i in range(4)]
        vaug = [kvp.tile([128, NKV * (HD + 1)], f32r, tag=f"va{i}",
                         name=f"va{i}") for i in range(NSLOT)]
        for g in range(NG):
            xb_t = [xin.tile([128, S], f32r, tag=f"xi{i}", name=f"xb{i}")
                    for i in range(8)]
            for i in range(8):
                nc.sync.dma_start(xb_t[i][:], xT_b[bass.ts(i, 128),
                                                   bass.ts(g, S)])
            cbg = ropeb.tile([128, S], f32, tag="cbg", name="cbg")
            nc.sync.dma_start(cbg[:], cosR_b[:, bass.ts(g, S)])
            sbg = ropeb.tile([128, S], f32, tag="sbg", name="sbg")
            nc.sync.dma_start(sbg[:], sinR_b[:, bass.ts(g, S)])
            bc_b = rms_stats([t[:].bitcast(f32) for t in xb_t], S, C)
            xn_b = []
            for i in range(8):
                xr = xb_t[i][:]
                nc.vector.tensor_mul(xr, xr.bitcast(f32), bc_b[:])
                xn_b.append(xr)
            for dk in range(4):
                k_ps = ps_acc.tile([128, S], f32, tag="acc", name="k_ps")
                wsl = wstream.tile([128, C], f32r, tag="wk", name="wk_sl",
                                   bufs=2)
                nc.sync.dma_start(
                    wsl[:].rearrange("p (a m) -> p a m", m=128),
                    wkT[:, bass.ts(dk, 128)].rearrange("(a p) m -> p a m",
                                                       p=128))
                for ci in range(8):
                    nc.tensor.matmul(k_ps[:], wsl[:, bass.ts(ci, 128)],
                                     xn_b[ci], start=(ci == 0), stop=(ci == 7))
                rope_norm(k_ps, cbg[:], sbg[:], S, kT[dk], g * S)
            for tt in range(4):
                slot = g * 4 + tt
                v_ps = ps_acc.tile([128, NKV * HD], f32, tag="acc", name="v_ps")
                for ci in range(8):
                    nc.tensor.matmul(v_ps[:],
                                     xn_b[ci][:, bass.ts(tt, 128)],
                                     wv_t[ci][:], start=(ci == 0), stop=(ci == 7))
                va = vaug[slot]
                va3 = va[:].rearrange("p (h d) -> p h d", d=HD + 1)
                vp3 = v_ps[:].rearrange("p (h d) -> p h d", d=HD)
                nc.scalar.copy(va3[:, :, 0:HD], vp3[:, :, :])
                nc.vector.tensor_copy(va3[:, :, HD:HD + 1], onescols_r[:])

        # ================= attention =================
        yT = [ypool.tile([128, S], f32r, tag=f"yT{i}", name=f"yT{i}")
              for i in range(8)]
        for h in range(NH):
            kh = h // 2                       # kv head
            dk, kp0 = kh // 2, 64 * (kh % 2)  # kT chunk/partition offset
            # q head layout is host-permuted so its partition base matches
            # the kv head base (matmul requires equal bases)
            dq, qp0 = 2 * (h // 4) + (h % 2), 64 * ((h // 2) % 2)
            assert qp0 == kp0
            yv = ps_att.tile([HD + 1, S], f32, tag="yv", name="yv")
            for s in range(NSLOT):
                s_ps = ps_att.tile([128, S], f32, tag="sps", name="s_ps")
                nc.tensor.matmul(
                    s_ps[:], kT[dk][kp0:kp0 + 64, bass.ts(s, 128)],
                    qT[dq][qp0:qp0 + 64, :], start=True, stop=True)
                pT = pexp.tile([128, S], f32r, tag="pT", name="pT")
                nc.scalar.activation(pT[:], s_ps[:], AF.Exp,
                                     bias=kb[:, s:s + 1], scale=0.125)
                if s < 4:
                    nc.gpsimd.affine_select(
                        pT[:], pT[:], pattern=[[1, S]], base=-128 * s,
                        channel_multiplier=-1,
                        compare_op=mybir.AluOpType.is_ge, fill=0.0)
                    if window < 512 - 128 * s:
                        nc.gpsimd.affine_select(
                            pT[:], pT[:], pattern=[[1, S]],
                            base=-128 * s - window, channel_multiplier=-1,
                            compare_op=mybir.AluOpType.is_le, fill=0.0)
                else:
                    m = NSLOT - s
                    if 128 * m - 127 <= window < 128 * m + 511:
                        nc.gpsimd.affine_select(
                            pT[:], pT[:], pattern=[[1, S]],
                            base=128 * m - window, channel_multiplier=-1,
                            compare_op=mybir.AluOpType.is_le, fill=0.0)
                nc.tensor.matmul(yv[:], vaug[s][:, 65 * kh:65 * kh + 65],
                                 pT[:], start=(s == 0), stop=(s == NSLOT - 1))
            ry = rows.tile([1, S], f32r, tag="ry", name="ry")
            with nc.allow_low_precision(reason="f32r softmax denom row"):
                nc.vector.reciprocal(ry[:], yv[HD:HD + 1, :])
            bc_y = ps_bc.tile([64, S], f32, tag="bc", name="bc_y")
            nc.tensor.matmul(bc_y[:], ones_row[:, :64], ry[:],
                             start=True, stop=True)
            bcy_s = work.tile([128, S], f32, tag="hbc", name="bcy")
            nc.scalar.copy(bcy_s[qp0:qp0 + 64, :], bc_y[:])
            nc.vector.tensor_mul(yT[dq][qp0:qp0 + 64, :], yv[0:HD, :],
                                 bcy_s[qp0:qp0 + 64, :])

        # ================= wo + residual + xf + router =================
        x2w = []
        for co in range(8):
            at_ps = ps_acc.tile([128, S], f32, tag="acc", name="at_ps")
            wsl = wstream.tile([128, C], f32r, tag="wo", name="wo_sl", bufs=2)
            nc.sync.dma_start(
                wsl[:].rearrange("p (a m) -> p a m", m=128),
                woT[:, bass.ts(co, 128)].rearrange("(a p) m -> p a m", p=128))
            for ci in range(8):
                nc.tensor.matmul(at_ps[:], wsl[:, bass.ts(ci, 128)],
                                 yT[ci][:], start=(ci == 0), stop=(ci == 7))
            xs2 = xin.tile([128, S], f32r, tag=f"xi{co}", name=f"xs2_{co}")
            nc.sync.dma_start(xs2[:], xT_s[bass.ts(co, 128), :])
            x2 = x2p.tile([128, S], f32, tag="x2w", name="x2w")
            nc.vector.tensor_add(x2[:], at_ps[:], xs2[:].bitcast(f32))
            nc.sync.dma_start(x2_out[bass.ts(co, 128), :], x2[:])
            x2w.append(x2)
        # xf stats need all 8 chunks: re-read x2 from DRAM to save SBUF
        x2r = [xin.tile([128, S], f32, tag=f"xi{i}", name=f"x2r{i}")
               for i in range(8)]
        for i in range(8):
            nc.sync.dma_start(x2r[i][:], x2_out[bass.ts(i, 128), :])
        bc_f = rms_stats(x2r, S, C)
        rt_ps = ps_row.tile([E_MLP + E_VE, S], f32, tag="row", name="rt_ps")
        for i in range(8):
            xf = x2p.tile([128, S], f32r, tag="xf", name="xf")
            nc.vector.tensor_mul(xf[:], x2r[i][:], bc_f[:])
            nc.sync.dma_start(xf_out[bass.ts(i, 128), :], xf[:])
            nc.tensor.matmul(rt_ps[:], rw_t[i][:], xf[:].bitcast(f32),
                             start=(i == 0), stop=(i == 7))
        lg = rows.tile([E_MLP + E_VE, S], f32, tag="lg", name="lg", bufs=1)
        nc.scalar.copy(lg[:], rt_ps[:])
        nc.sync.dma_start(logit_out[:], lg[:])

    nc.compile()
    return nc


# --------------------------------------------------------------------------
# Phase 2 builder: sparse expert MLP + VE weighting
# --------------------------------------------------------------------------
def build_phase2(ncap: int):
    nc = bacc.Bacc("TRN2", target_bir_lowering=False, debug=False,
                   num_devices=NCORES)
    NT = ncap // 256

    xfg = nc.dram_tensor("xfg", [C, ncap], f32r, kind="ExternalInput").ap()
    w_upT = nc.dram_tensor("w_upT", [C, HID], f32r, kind="ExternalInput").ap()
    w_downT = nc.dram_tensor("w_downT", [HID, C], f32r,
                             kind="ExternalInput").ap()
    gate = nc.dram_tensor("gate", [1, ncap], f32, kind="ExternalInput").ap()
    ve0 = nc.dram_tensor("ve0", [S, C], f32, kind="ExternalInput").ap()
    ve1 = nc.dram_tensor("ve1", [S, C], f32, kind="ExternalInput").ap()
    ve_g = nc.dram_tensor("ve_g", [128, 8], f32, kind="ExternalInput").ap()

    moe_out = nc.dram_tensor("moe_out", [C, ncap], f32, kind="ExternalOutput").ap()
    ve_out = nc.dram_tensor("ve_out", [S, C], f32, kind="ExternalOutput").ap()

    from contextlib import ExitStack
    with tile.TileContext(nc) as tc, ExitStack() as est:
        const = est.enter_context(tc.tile_pool(name="const", bufs=1))
        wpool = est.enter_context(tc.tile_pool(name="wpool", bufs=1))
        hpool = est.enter_context(tc.tile_pool(name="hpool", bufs=1))
        stream = est.enter_context(tc.tile_pool(name="stream", bufs=2))
        work = est.enter_context(tc.tile_pool(name="work", bufs=2))
        ps_h = est.enter_context(tc.tile_pool(name="ps_h", bufs=2, space="PSUM"))
        ps_o = est.enter_context(tc.tile_pool(name="ps_o", bufs=2, space="PSUM"))
        ps_b = est.enter_context(tc.tile_pool(name="ps_b", bufs=2, space="PSUM"))

        ones_row = const.tile([1, 128], f32)
        nc.vector.memset(ones_row[:], 1.0)

        up_t = [wpool.tile([128, HID], f32r, tag=f"up{i}", name=f"up{i}") for i in range(8)]
        dn_t = [wpool.tile([128, C], f32r, tag=f"dn{i}", name=f"dn{i}") for i in range(16)]
        for i in range(8):
            nc.sync.dma_start(up_t[i][:], w_upT[bass.ts(i, 128), :])
        for i in range(16):
            nc.sync.dma_start(dn_t[i][:], w_downT[bass.ts(i, 128), :])
        veg = const.tile([128, 8], f32)
        nc.sync.dma_start(veg[:], ve_g[:])
        gate_sb = const.tile([1, ncap], f32)
        nc.sync.dma_start(gate_sb[:], gate[:])

        for nt in range(NT):
            csl = bass.ts(nt, 256)
            xf_t = [stream.tile([128, 256], f32r, tag=f"xf{i}", name=f"xf{i}")
                    for i in range(8)]
            for i in range(8):
                nc.sync.dma_start(xf_t[i][:], xfg[bass.ts(i, 128), csl])
            g_ps = ps_b.tile([128, 256], f32)
            nc.tensor.matmul(g_ps[:], ones_row[:], gate_sb[:, csl],
                             start=True, stop=True)
            g_bc = work.tile([128, 256], f32, tag="gbc", name="gbc")
            nc.scalar.copy(g_bc[:], g_ps[:])
            hT = [hpool.tile([128, 256], f32r, tag=f"hT{i}", name=f"hT{i}")
                  for i in range(16)]
            for hc in range(16):
                h_ps = ps_h.tile([128, 256], f32)
                for ci in range(8):
                    nc.tensor.matmul(h_ps[:], up_t[ci][:, bass.ts(hc, 128)],
                                     xf_t[ci][:], start=(ci == 0),
                                     stop=(ci == 7))
                hr = work.tile([128, 256], f32, tag="hrelu", name="hrelu")
                nc.scalar.activation(hr[:], h_ps[:], AF.Relu)
                nc.vector.tensor_mul(hT[hc][:], hr[:], hr[:])
            for co in range(8):
                o_ps = ps_o.tile([128, 256], f32)
                for hc in range(16):
                    nc.tensor.matmul(o_ps[:], dn_t[hc][:, bass.ts(co, 128)],
                                     hT[hc][:], start=(hc == 0),
                                     stop=(hc == 15))
                ot = work.tile([128, 256], f32, tag="ot", name="ot")
                nc.vector.tensor_mul(ot[:], o_ps[:], g_bc[:])
                nc.sync.dma_start(moe_out[bass.ts(co, 128), csl], ot[:])

        # VE weighting for own token strip (token-major)
        for tt in range(4):
            rsl = bass.ts(tt, 128)
            r0 = stream.tile([128, C], f32, tag="ver0", name="ver0")
            r1 = stream.tile([128, C], f32, tag="ver1", name="ver1")
            nc.sync.dma_start(r0[:], ve0[rsl, :])
            nc.sync.dma_start(r1[:], ve1[rsl, :])
            nc.vector.tensor_scalar_mul(r0[:], r0[:], veg[:, 2 * tt:2 * tt + 1])
            nc.vector.tensor_scalar_mul(r1[:], r1[:],
                                        veg[:, 2 * tt + 1:2 * tt + 2])
            nc.vector.tensor_add(r0[:], r0[:], r1[:])
            nc.sync.dma_start(ve_out[rsl, :], r0[:])

    nc.compile()
    return nc


# --------------------------------------------------------------------------
# Host orchestration
# --------------------------------------------------------------------------
def _phase1_inputs(x, cos, sin, window, wq, wk, wv, wo, router_w):
    """Build per-core in_maps for phase 1."""
    cosT = np.ascontiguousarray(cos[0, :, 0, :].T)  # (32, T)
    sinT = np.ascontiguousarray(sin[0, :, 0, :].T)
    cosR = np.tile(cosT, (4, 1)).astype(np.float32)          # (128, T)
    sinR = np.tile(np.vstack([sinT, -sinT]), (2, 1)).astype(np.float32)

    # q-head placement permutation (see attention loop): head h lives at
    # chunk 2*(h//4)+(h%2), partition base 64*((h//2)%2)
    colmap = np.zeros(NH * HD, np.int64)
    for h in range(NH):
        pos = (2 * (h // 4) + (h % 2)) * 128 + 64 * ((h // 2) % 2)
        colmap[pos:pos + HD] = np.arange(h * HD, (h + 1) * HD)
    wqT = np.ascontiguousarray(wq.T[:, colmap])
    wkT = np.ascontiguousarray(wk.T)
    wvT = np.ascontiguousarray(wv.T)
    woT = np.ascontiguousarray(wo.T[colmap, :])
    rwT = np.ascontiguousarray(router_w.T)

    in_maps = []
    perms = []
    for c in range(NCORES):
        b, qi = c // 4, c % 4
        qoff = S * qi
        q128 = qoff // 128
        perm = [(q128 + s) % NSLOT for s in range(NSLOT)]
        perms.append(perm)
        xT = x[b].T  # (C, T)
        xT_rot = np.ascontiguousarray(
            xT.reshape(C, NSLOT, 128)[:, perm, :].reshape(C, T))
        cosR_b = np.ascontiguousarray(
            cosR.reshape(128, NSLOT, 128)[:, perm, :].reshape(128, T))
        sinR_b = np.ascontiguousarray(
            sinR.reshape(128, NSLOT, 128)[:, perm, :].reshape(128, T))
        # per-slot alive bias
        kbias = np.zeros((128, NSLOT), np.float32)
        for s in range(NSLOT):
            kt = perm[s]
            # any (q in [qoff, qoff+511], k in [kt*128, kt*128+127]) with
            # k <= q and q - k <= window?
            dmin = qoff - (kt * 128 + 127)
            dmax = qoff + S - 1 - kt * 128
            alive = (dmax >= 0) and (dmin <= window)
            if not alive:
                kbias[:, s] = -30000.0
        in_maps.append(dict(
            xT_b=xT_rot,
            cosR_b=cosR_b, sinR_b=sinR_b, kbias=kbias,
            wqT=wqT, wkT=wkT, wvT=wvT, woT=woT, rwT=rwT,
        ))
    return in_maps, perms


def _route(logits, router_bias):
    """Top-2 routing exactly as the reference (on host, f32)."""
    sig = (1.0 / (1.0 + np.exp(-logits.astype(np.float32)))).astype(np.float32)
    sel = sig + router_bias[None, :].astype(np.float32)
    idx = np.argsort(-sel, axis=1, kind="stable")[:, :TOPK]
    tw = np.take_along_axis(sig, idx, axis=1)
    tw = tw / tw.sum(axis=1, keepdims=True)
    N = logits.shape[0]
    sparse_w = np.zeros((N, E_MLP + E_VE), np.float32)
    np.put_along_axis(sparse_w, idx, tw, axis=1)
    return sparse_w


def kernel(**inputs):
    x = np.asarray(inputs["x"], np.float32)
    token_ids = np.asarray(inputs["token_ids"])
    cos = np.asarray(inputs["cos"], np.float32)
    sin = np.asarray(inputs["sin"], np.float32)
    window = int(np.asarray(inputs["window_size"]))
    wq, wk, wv, wo = (np.asarray(inputs[k], np.float32)
                      for k in ("wq", "wk", "wv", "wo"))
    w_up = np.asarray(inputs["w_up"], np.float32)
    w_down = np.asarray(inputs["w_down"], np.float32)
    router_w = np.asarray(inputs["router_w"], np.float32)
    router_bias = np.asarray(inputs["router_bias"], np.float32)
    ve_tables = np.asarray(inputs["ve_tables"], np.float32)

    key1 = ("p1", window)
    if key1 not in _prog_cache:
        _prog_cache[key1] = build_phase1(window)
    nc1 = _prog_cache[key1]

    in_maps, _ = _phase1_inputs(x, cos, sin, window, wq, wk, wv, wo, router_w)
    res1 = run_bass_kernel_spmd(nc1, in_maps, list(range(NCORES))).results

    x2T = np.concatenate([res1[c]["x2_out"] for c in range(NCORES)], axis=1)
    xfT = np.concatenate([res1[c]["xf_out"] for c in range(NCORES)], axis=1)
    logits = np.concatenate([res1[c]["logit_out"].T for c in range(NCORES)],
                            axis=0)  # (N, 10)

    N = B * T
    sparse_w = _route(logits, router_bias)

    # dispatch
    ncap = NCAP
    idx_list, n_list = [], []
    for e in range(E_MLP):
        idx_e = np.nonzero(sparse_w[:, e])[0]
        idx_list.append(idx_e)
        n_list.append(len(idx_e))
    max_n = max(n_list)
    while ncap < max_n:
        ncap *= 2

    key2 = ("p2", ncap)
    if key2 not in _prog_cache:
        _prog_cache[key2] = build_phase2(ncap)
    nc2 = _prog_cache[key2]

    tok = token_ids.reshape(-1)
    in_maps2 = []
    for c in range(NCORES):
        e = c
        idx_e = idx_list[e]
        xfg = np.zeros((C, ncap), np.float32)
        xfg[:, :n_list[e]] = xfT[:, idx_e]
        gate = np.zeros((1, ncap), np.float32)
        gate[0, :n_list[e]] = sparse_w[idx_e, e]
        s0 = c * S
        strip_tok = tok[s0:s0 + S]
        ve0 = np.ascontiguousarray(ve_tables[0][strip_tok])
        ve1 = np.ascontiguousarray(ve_tables[1][strip_tok])
        veg = np.zeros((128, 8), np.float32)
        for tt in range(4):
            for ee in range(E_VE):
                veg[:, 2 * tt + ee] = sparse_w[s0 + tt * 128:s0 + (tt + 1) * 128,
                                               E_MLP + ee]
        in_maps2.append(dict(
            xfg=xfg, w_upT=np.ascontiguousarray(w_up[e].T),
            w_downT=np.ascontiguousarray(w_down[e].T),
            gate=gate, ve0=ve0, ve1=ve1, ve_g=veg,
        ))
    res2 = run_bass_kernel_spmd(nc2, in_maps2, list(range(NCORES))).results

    out = np.ascontiguousarray(x2T.T)  # (N, C)
    for c in range(NCORES):
        out[c * S:(c + 1) * S] += res2[c]["ve_out"]
    for e in range(E_MLP):
        n_e = n_list[e]
        if n_e:
            out[idx_list[e]] += res2[e]["moe_out"][:, :n_e].T
    return out.reshape(B, T, C).astype(np.float32)


# revision 29
# speedup vs baseline: 13039.7077x; 1.0015x over previous
"""TRN2 Bass kernel for nn_BlockMoVaE (attention + MoE/VE routing block).

Self-contained: accepts FULL inputs, shards across 8 NeuronCores, returns
FULL output.

Sharding:
  Phase 1 (attention + router logits): token-parallel. Core c handles the
    512-query strip [qoff, qoff+512) of batch b=c//4, qoff=512*(c%4).
    Activations are kept FEATURE-major ([feature, token]) so no on-device
    transposes are needed. K/V are computed for the whole batch on each
    core of the batch group, with key 128-tiles stored in a per-core
    ROTATED slot order (slot s holds absolute key tile (qoff/128+s)%16) so
    the causal boundary lands at static slots 0..3 in every core's
    (shared, SPMD) program; fully-masked future tiles are killed by a
    per-slot additive bias (-3e4) inside the exp activation.
  Phase 2 (expert-parallel sparse MoE): core e computes MLP expert e over
    only the tokens routed to it (host gathers columns, capacity-padded);
    VE (vocab-embedding expert) rows are host-gathered and weighted on
    device per token strip. Host does top-2 routing between launches and
    the final scatter-add/assembly.

Matmuls run as float32r (full PE rate, ~1e-4 rel err); PSUM accumulates
in fp32.
"""
import numpy as np

import concourse.bass as bass
import concourse.bacc as bacc
import concourse.mybir as mybir
import concourse.tile as tile
from concourse.bass_utils import run_bass_kernel_spmd

# ---- problem constants (hardcoded per contest rules) ----
B, T, C = 2, 2048, 1024
NH, NKV, HD = 16, 8, 64
E_MLP, E_VE, TOPK = 8, 2, 2
HID = 2048
VOCAB = 50257
EPS = 1e-6
NCORES = 8
S = 512            # tokens per core strip
NSLOT = T // 128   # 16 key tiles per batch
NG = 4             # kv column groups of 512
NCAP = 1024        # expert token capacity (phase 2)

f32 = mybir.dt.float32
f32r = mybir.dt.float32r
bf16 = mybir.dt.bfloat16
AF = mybir.ActivationFunctionType

_prog_cache = {}



def _register_consts(nc, values):
    for value in values:
        key = (f32, float(value))
        if key not in nc.const_aps.aps:
            t = nc.alloc_sbuf_tensor(f"constap-{value}", [128, 1], f32)
            nc.gpsimd.memset(t.ap(), float(value))
            nc.const_aps.aps[key] = t.ap()
    nc.all_engine_barrier()


# --------------------------------------------------------------------------
# Phase 1 builder: attention + residual + rmsnorm + router logits
# --------------------------------------------------------------------------
def build_phase1(window: int):
    nc = bacc.Bacc("TRN2", target_bir_lowering=False, debug=False,
                   num_devices=NCORES)

    xT_b = nc.dram_tensor("xT_b", [C, T], f32r, kind="ExternalInput").ap()
    xT_s = xT_b[:, 0:S]          # strip == rotated slots 0..3
    cosR_b = nc.dram_tensor("cosR_b", [128, T], f32, kind="ExternalInput").ap()
    sinR_b = nc.dram_tensor("sinR_b", [128, T], f32, kind="ExternalInput").ap()
    cosR_s = cosR_b[:, 0:S]
    sinR_s = sinR_b[:, 0:S]
    kbias = nc.dram_tensor("kbias", [128, NSLOT], f32, kind="ExternalInput").ap()
    wqT = nc.dram_tensor("wqT", [C, NH * HD], f32r, kind="ExternalInput").ap()
    wkT = nc.dram_tensor("wkT", [C, NKV * HD], f32r, kind="ExternalInput").ap()
    wvT = nc.dram_tensor("wvT", [C, NKV * HD], f32r, kind="ExternalInput").ap()
    woT = nc.dram_tensor("woT", [C, C], f32r, kind="ExternalInput").ap()
    rwT = nc.dram_tensor("rwT", [C, E_MLP + E_VE], f32, kind="ExternalInput").ap()

    x2_out = nc.dram_tensor("x2_out", [C, S], f32, kind="ExternalOutput").ap()
    xf_out = nc.dram_tensor("xf_out", [C, S], f32r, kind="ExternalOutput").ap()
    logit_out = nc.dram_tensor("logit_out", [E_MLP + E_VE, S], f32,
                               kind="ExternalOutput").ap()

    _register_consts(nc, [EPS])
    from contextlib import ExitStack
    with tile.TileContext(nc) as tc, ExitStack() as est:
        const = est.enter_context(tc.tile_pool(name="const", bufs=1))
        ropes = est.enter_context(tc.tile_pool(name="ropes", bufs=1))
        ropeb = est.enter_context(tc.tile_pool(name="ropeb", bufs=1))
        wstream = est.enter_context(tc.tile_pool(name="wstream", bufs=3))
        wvp = est.enter_context(tc.tile_pool(name="wvp", bufs=1))
        xin = est.enter_context(tc.tile_pool(name="xin", bufs=1))
        kvp = est.enter_context(tc.tile_pool(name="kv", bufs=1))
        qp = est.enter_context(tc.tile_pool(name="qp", bufs=1))
        work = est.enter_context(tc.tile_pool(name="work", bufs=2))
        rows = est.enter_context(tc.tile_pool(name="rows", bufs=2))
        pexp = est.enter_context(tc.tile_pool(name="pexp", bufs=2))
        ypool = est.enter_context(tc.tile_pool(name="ypool", bufs=1))
        x2p = est.enter_context(tc.tile_pool(name="x2p", bufs=2))
        ps_acc = est.enter_context(tc.tile_pool(name="ps_acc", bufs=2, space="PSUM"))
        ps_row = est.enter_context(tc.tile_pool(name="ps_row", bufs=1, space="PSUM"))
        ps_bc = est.enter_context(tc.tile_pool(name="ps_bc", bufs=1, space="PSUM"))
        ps_att = est.enter_context(tc.tile_pool(name="ps_att", bufs=2, space="PSUM"))

        # ---- constants ----
        ones_col_f = const.tile([128, 1], f32, name="ones_col_f")
        nc.vector.memset(ones_col_f[:], 1.0)
        ones_col = const.tile([128, 1], f32r, name="ones_col")
        nc.scalar.copy(ones_col[:], ones_col_f[:])
        ones_row_f = const.tile([1, 128], f32, name="ones_row_f")
        nc.vector.memset(ones_row_f[:], 1.0)
        ones_row = const.tile([1, 128], f32r, name="ones_row")
        nc.scalar.copy(ones_row[:], ones_row_f[:])
        onescols = const.tile([128, NKV, 1], f32, name="onescols")
        nc.vector.memset(onescols[:], 1.0)
        onescols_r = const.tile([128, NKV, 1], f32r, name="onescols_r")
        nc.vector.tensor_copy(onescols_r[:], onescols[:])
        kb = const.tile([128, NSLOT], f32, name="kb")
        nc.sync.dma_start(kb[:], kbias[:])

        cs = ropes.tile([128, S], f32, name="cs")
        nc.sync.dma_start(cs[:], cosR_s[:])
        ss = ropes.tile([128, S], f32, name="ss")
        nc.sync.dma_start(ss[:], sinR_s[:])

        rw_t = [const.tile([128, E_MLP + E_VE], f32, tag=f"rw{i}",
                           name=f"rw{i}") for i in range(8)]
        wv_t = [wvp.tile([128, NKV * HD], f32r, tag=f"wv{i}", name=f"wv{i}")
                for i in range(8)]
        for i in range(8):
            nc.sync.dma_start(rw_t[i][:], rwT[bass.ts(i, 128), :])
            nc.sync.dma_start(wv_t[i][:], wvT[bass.ts(i, 128), :])

        # ---- helper: rms broadcast for feature-major tiles ----
        def rms_stats(xtiles, n, nfeat):
            ssq = ps_row.tile([1, n], f32, tag="row", name="ssq")
            for i, xt in enumerate(xtiles):
                sq = work.tile([128, n], f32r, tag="sqstat", name="sqstat")
                nc.vector.tensor_mul(sq[:], xt[:], xt[:])
                nc.tensor.matmul(ssq[:], ones_col[:], sq[:],
                                 start=(i == 0), stop=(i == len(xtiles) - 1))
            srow = rows.tile([1, n], f32, tag="srow", name="srow")
            nc.scalar.activation(srow[:], ssq[:], AF.Sqrt,
                                 bias=EPS, scale=1.0 / nfeat)
            rrow = rows.tile([1, n], f32r, tag="rrow", name="rrow")
            with nc.allow_low_precision(reason="f32r rms bcast rows"):
                nc.vector.reciprocal(rrow[:], srow[:])
            bc = ps_bc.tile([128, n], f32, tag="bc", name="bc")
            nc.tensor.matmul(bc[:], ones_row[:], rrow[:], start=True, stop=True)
            bcs = work.tile([128, n], f32, tag="bcstat", name="bcstat", bufs=1)
            nc.scalar.copy(bcs[:], bc[:])
            return bcs

        # ---- helper: rope + per-head rmsnorm on a projection psum ----
        def rope_norm(ps, cos_ap, sin_ap, n, out_tile, col0):
            # swp[blk] = ps[swapped 32-block] * sinR[blk] (psum offset reads)
            swp = work.tile([128, n], f32, tag="swp", name="swp")
            for blk in range(4):
                sb0 = (blk // 2) * 64 + (1 - blk % 2) * 32
                b0 = blk * 32
                nc.vector.tensor_mul(swp[b0:b0 + 32, :], ps[sb0:sb0 + 32, :],
                                     sin_ap[b0:b0 + 32, :])
            t1 = work.tile([128, n], f32, tag="ropet1", name="ropet1")
            nc.vector.tensor_mul(t1[:], ps[:], cos_ap)
            nc.vector.tensor_add(swp[:], t1[:], swp[:])   # roped value
            sq = work.tile([128, n], f32r, tag="ropet1", name="ropesq")
            nc.vector.tensor_mul(sq[:], swp[:], swp[:])
            for hh in range(2):
                p0 = 64 * hh
                ssqh = ps_row.tile([1, n], f32, tag="row", name="ssqh")
                nc.tensor.matmul(ssqh[:], ones_col[p0:p0 + 64, :],
                                 sq[p0:p0 + 64, :], start=True, stop=True)
                srow = rows.tile([1, n], f32, tag="srow", name="hsrow")
                nc.scalar.activation(srow[:], ssqh[:], AF.Sqrt,
                                     bias=EPS, scale=1.0 / HD)
                rrow = rows.tile([1, n], f32r, tag="rrow", name="hrrow")
                with nc.allow_low_precision(reason="f32r rms bcast rows"):
                    nc.vector.reciprocal(rrow[:], srow[:])
                bch = ps_bc.tile([64, n], f32, tag="bc", name="bch")
                nc.tensor.matmul(bch[:], ones_row[:, :64], rrow[:],
                                 start=True, stop=True)
                bcs = work.tile([128, n], f32, tag="hbc", name="hbc")
                nc.scalar.copy(bcs[p0:p0 + 64, :], bch[:])
                nc.vector.tensor_mul(
                    out_tile[p0:p0 + 64, col0:col0 + n],
                    swp[p0:p0 + 64, :], bcs[p0:p0 + 64, :])

        # ================= strip pipeline (Q) =================
        xs_t = [xin.tile([128, S], f32r, tag=f"xi{i}", name=f"xs{i}")
                for i in range(8)]
        for i in range(8):
            nc.sync.dma_start(xs_t[i][:], xT_s[bass.ts(i, 128), :])
        bc_s = rms_stats([t[:].bitcast(f32) for t in xs_t], S, C)
        xn_s = []
        for i in range(8):
            xr = xs_t[i][:]
            nc.vector.tensor_mul(xr, xr.bitcast(f32), bc_s[:])  # in-place norm
            xn_s.append(xr)

        qT = [qp.tile([128, S], f32r, tag=f"qT{i}", name=f"qT{i}")
              for i in range(8)]
        for dq in range(8):
            q_ps = ps_acc.tile([128, S], f32, tag="acc", name="q_ps")
            wsl = wstream.tile([128, C], f32r, tag="wq", name="wq_sl", bufs=2)
            nc.sync.dma_start(
                wsl[:].rearrange("p (a m) -> p a m", m=128),
                wqT[:, bass.ts(dq, 128)].rearrange("(a p) m -> p a m", p=128))
            for ci in range(8):
                nc.tensor.matmul(q_ps[:], wsl[:, bass.ts(ci, 128)], xn_s[ci],
                                 start=(ci == 0), stop=(ci == 7))
            rope_norm(q_ps, cs[:], ss[:], S, qT[dq], 0)

        # ================= batch pipeline (K, V) =================
        kT = [kvp.tile([128, T], f32r, tag=f"kT{i}", name=f"kT{i}")
              for i in range(4)]
        vaug = [kvp.tile([128, NKV * (HD + 1)], f32r, tag=f"va{i}",
                         name=f"va{i}") for i in range(NSLOT)]
        for g in range(NG):
            xb_t = [xin.tile([128, S], f32r, tag=f"xi{i}", name=f"xb{i}")
                    for i in range(8)]
            for i in range(8):
                nc.sync.dma_start(xb_t[i][:], xT_b[bass.ts(i, 128),
                                                   bass.ts(g, S)])
            cbg = ropeb.tile([128, S], f32, tag="cbg", name="cbg")
            nc.sync.dma_start(cbg[:], cosR_b[:, bass.ts(g, S)])
            sbg = ropeb.tile([128, S], f32, tag="sbg", name="sbg")
            nc.sync.dma_start(sbg[:], sinR_b[:, bass.ts(g, S)])
            bc_b = rms_stats([t[:].bitcast(f32) for t in xb_t], S, C)
            xn_b = []
            for i in range(8):
                xr = xb_t[i][:]
                nc.vector.tensor_mul(xr, xr.bitcast(f32), bc_b[:])
                xn_b.append(xr)
            for dk in range(4):
                k_ps = ps_acc.tile([128, S], f32, tag="acc", name="k_ps")
                wsl = wstream.tile([128, C], f32r, tag="wk", name="wk_sl",
                                   bufs=2)
                nc.sync.dma_start(
                    wsl[:].rearrange("p (a m) -> p a m", m=128),
                    wkT[:, bass.ts(dk, 128)].rearrange("(a p) m -> p a m",
                                                       p=128))
                for ci in range(8):
                    nc.tensor.matmul(k_ps[:], wsl[:, bass.ts(ci, 128)],
                                     xn_b[ci], start=(ci == 0), stop=(ci == 7))
                rope_norm(k_ps, cbg[:], sbg[:], S, kT[dk], g * S)
            for tt in range(4):
                slot = g * 4 + tt
                v_ps = ps_acc.tile([128, NKV * HD], f32, tag="acc", name="v_ps")
                for ci in range(8):
                    nc.tensor.matmul(v_ps[:],
                                     xn_b[ci][:, bass.ts(tt, 128)],
                                     wv_t[ci][:], start=(ci == 0), stop=(ci == 7))
                va = vaug[slot]
                va3 = va[:].rearrange("p (h d) -> p h d", d=HD + 1)
                vp3 = v_ps[:].rearrange("p (h d) -> p h d", d=HD)
                nc.scalar.copy(va3[:, :, 0:HD], vp3[:, :, :])
                nc.vector.tensor_copy(va3[:, :, HD:HD + 1], onescols_r[:])

        # ================= attention =================
        yT = [ypool.tile([128, S], f32r, tag=f"yT{i}", name=f"yT{i}")
              for i in range(8)]
        for h in range(NH):
            kh = h // 2                       # kv head
            dk, kp0 = kh // 2, 64 * (kh % 2)  # kT chunk/partition offset
            # q head layout is host-permuted so its partition base matches
            # the kv head base (matmul requires equal bases)
            dq, qp0 = 2 * (h // 4) + (h % 2), 64 * ((h // 2) % 2)
            assert qp0 == kp0
            yv = ps_att.tile([HD + 1, S], f32, tag="yv", name="yv")
            for s in range(NSLOT):
                s_ps = ps_att.tile([128, S], f32, tag="sps", name="s_ps")
                nc.tensor.matmul(
                    s_ps[:], kT[dk][kp0:kp0 + 64, bass.ts(s, 128)],
                    qT[dq][qp0:qp0 + 64, :], start=True, stop=True)
                pT = pexp.tile([128, S], f32r, tag="pT", name="pT")
                nc.scalar.activation(pT[:], s_ps[:], AF.Exp,
                                     bias=kb[:, s:s + 1], scale=0.125)
                if s < 4:
                    nc.gpsimd.affine_select(
                        pT[:], pT[:], pattern=[[1, S]], base=-128 * s,
                        channel_multiplier=-1,
                        compare_op=mybir.AluOpType.is_ge, fill=0.0)
                    if window < 512 - 128 * s:
                        nc.gpsimd.affine_select(
                            pT[:], pT[:], pattern=[[1, S]],
                            base=-128 * s - window, channel_multiplier=-1,
                            compare_op=mybir.AluOpType.is_le, fill=0.0)
                else:
                    m = NSLOT - s
                    if 128 * m - 127 <= window < 128 * m + 511:
                        nc.gpsimd.affine_select(
                            pT[:], pT[:], pattern=[[1, S]],
                            base=128 * m - window, channel_multiplier=-1,
                            compare_op=mybir.AluOpType.is_le, fill=0.0)
                nc.tensor.matmul(yv[:], vaug[s][:, 65 * kh:65 * kh + 65],
                                 pT[:], start=(s == 0), stop=(s == NSLOT - 1))
            ry = rows.tile([1, S], f32r, tag="ry", name="ry")
            with nc.allow_low_precision(reason="f32r softmax denom row"):
                nc.vector.reciprocal(ry[:], yv[HD:HD + 1, :])
            bc_y = ps_bc.tile([64, S], f32, tag="bc", name="bc_y")
            nc.tensor.matmul(bc_y[:], ones_row[:, :64], ry[:],
                             start=True, stop=True)
            bcy_s = work.tile([128, S], f32, tag="hbc", name="bcy")
            nc.vector.tensor_copy(bcy_s[qp0:qp0 + 64, :], bc_y[:])
            nc.vector.tensor_mul(yT[dq][qp0:qp0 + 64, :], yv[0:HD, :],
                                 bcy_s[qp0:qp0 + 64, :])

        # ================= wo + residual + xf + router =================
        x2w = []
        for co in range(8):
            at_ps = ps_acc.tile([128, S], f32, tag="acc", name="at_ps")
            wsl = wstream.tile([128, C], f32r, tag="wo", name="wo_sl", bufs=2)
            nc.sync.dma_start(
                wsl[:].rearrange("p (a m) -> p a m", m=128),
                woT[:, bass.ts(co, 128)].rearrange("(a p) m -> p a m", p=128))
            for ci in range(8):
                nc.tensor.matmul(at_ps[:], wsl[:, bass.ts(ci, 128)],
                                 yT[ci][:], start=(ci == 0), stop=(ci == 7))
            xs2 = xin.tile([128, S], f32r, tag=f"xi{co}", name=f"xs2_{co}")
            nc.sync.dma_start(xs2[:], xT_s[bass.ts(co, 128), :])
            x2 = x2p.tile([128, S], f32, tag="x2w", name="x2w")
            nc.vector.tensor_add(x2[:], at_ps[:], xs2[:].bitcast(f32))
            nc.sync.dma_start(x2_out[bass.ts(co, 128), :], x2[:])
            x2w.append(x2)
        # xf stats need all 8 chunks: re-read x2 from DRAM to save SBUF
        x2r = [xin.tile([128, S], f32, tag=f"xi{i}", name=f"x2r{i}")
               for i in range(8)]
        for i in range(8):
            nc.sync.dma_start(x2r[i][:], x2_out[bass.ts(i, 128), :])
        bc_f = rms_stats(x2r, S, C)
        rt_ps = ps_row.tile([E_MLP + E_VE, S], f32, tag="row", name="rt_ps")
        for i in range(8):
            xf = x2p.tile([128, S], f32r, tag="xf", name="xf")
            nc.vector.tensor_mul(xf[:], x2r[i][:], bc_f[:])
            nc.sync.dma_start(xf_out[bass.ts(i, 128), :], xf[:])
            nc.tensor.matmul(rt_ps[:], rw_t[i][:], xf[:].bitcast(f32),
                             start=(i == 0), stop=(i == 7))
        lg = rows.tile([E_MLP + E_VE, S], f32, tag="lg", name="lg", bufs=1)
        nc.scalar.copy(lg[:], rt_ps[:])
        nc.sync.dma_start(logit_out[:], lg[:])

    nc.compile()
    return nc


# --------------------------------------------------------------------------
# Phase 2 builder: sparse expert MLP + VE weighting
# --------------------------------------------------------------------------
def build_phase2(ncap: int):
    nc = bacc.Bacc("TRN2", target_bir_lowering=False, debug=False,
                   num_devices=NCORES)
    NT = ncap // 256

    xfg = nc.dram_tensor("xfg", [C, ncap], f32r, kind="ExternalInput").ap()
    w_upT = nc.dram_tensor("w_upT", [C, HID], f32r, kind="ExternalInput").ap()
    w_downT = nc.dram_tensor("w_downT", [HID, C], f32r,
                             kind="ExternalInput").ap()
    gate = nc.dram_tensor("gate", [1, ncap], f32, kind="ExternalInput").ap()
    ve0 = nc.dram_tensor("ve0", [S, C], f32, kind="ExternalInput").ap()
    ve1 = nc.dram_tensor("ve1", [S, C], f32, kind="ExternalInput").ap()
    ve_g = nc.dram_tensor("ve_g", [128, 8], f32, kind="ExternalInput").ap()

    moe_out = nc.dram_tensor("moe_out", [C, ncap], f32, kind="ExternalOutput").ap()
    ve_out = nc.dram_tensor("ve_out", [S, C], f32, kind="ExternalOutput").ap()

    from contextlib import ExitStack
    with tile.TileContext(nc) as tc, ExitStack() as est:
        const = est.enter_context(tc.tile_pool(name="const", bufs=1))
        wpool = est.enter_context(tc.tile_pool(name="wpool", bufs=1))
        hpool = est.enter_context(tc.tile_pool(name="hpool", bufs=1))
        stream = est.enter_context(tc.tile_pool(name="stream", bufs=2))
        work = est.enter_context(tc.tile_pool(name="work", bufs=2))
        ps_h = est.enter_context(tc.tile_pool(name="ps_h", bufs=2, space="PSUM"))
        ps_o = est.enter_context(tc.tile_pool(name="ps_o", bufs=2, space="PSUM"))
        ps_b = est.enter_context(tc.tile_pool(name="ps_b", bufs=2, space="PSUM"))

        ones_row = const.tile([1, 128], f32)
        nc.vector.memset(ones_row[:], 1.0)

        up_t = [wpool.tile([128, HID], f32r, tag=f"up{i}", name=f"up{i}") for i in range(8)]
        dn_t = [wpool.tile([128, C], f32r, tag=f"dn{i}", name=f"dn{i}") for i in range(16)]
        for i in range(8):
            nc.sync.dma_start(up_t[i][:], w_upT[bass.ts(i, 128), :])
        for i in range(16):
            nc.sync.dma_start(dn_t[i][:], w_downT[bass.ts(i, 128), :])
        veg = const.tile([128, 8], f32)
        nc.sync.dma_start(veg[:], ve_g[:])
        gate_sb = const.tile([1, ncap], f32)
        nc.sync.dma_start(gate_sb[:], gate[:])

        for nt in range(NT):
            csl = bass.ts(nt, 256)
            xf_t = [stream.tile([128, 256], f32r, tag=f"xf{i}", name=f"xf{i}")
                    for i in range(8)]
            for i in range(8):
                nc.sync.dma_start(xf_t[i][:], xfg[bass.ts(i, 128), csl])
            g_ps = ps_b.tile([128, 256], f32)
            nc.tensor.matmul(g_ps[:], ones_row[:], gate_sb[:, csl],
                             start=True, stop=True)
            g_bc = work.tile([128, 256], f32, tag="gbc", name="gbc")
            nc.scalar.copy(g_bc[:], g_ps[:])
            hT = [hpool.tile([128, 256], f32r, tag=f"hT{i}", name=f"hT{i}")
                  for i in range(16)]
            for hc in range(16):
                h_ps = ps_h.tile([128, 256], f32)
                for ci in range(8):
                    nc.tensor.matmul(h_ps[:], up_t[ci][:, bass.ts(hc, 128)],
                                     xf_t[ci][:], start=(ci == 0),
                                     stop=(ci == 7))
                hr = work.tile([128, 256], f32, tag="hrelu", name="hrelu")
                nc.scalar.activation(hr[:], h_ps[:], AF.Relu)
                nc.vector.tensor_mul(hT[hc][:], hr[:], hr[:])
            for co in range(8):
                o_ps = ps_o.tile([128, 256], f32)
                for hc in range(16):
                    nc.tensor.matmul(o_ps[:], dn_t[hc][:, bass.ts(co, 128)],
                                     hT[hc][:], start=(hc == 0),
                                     stop=(hc == 15))
                ot = work.tile([128, 256], f32, tag="ot", name="ot")
                nc.vector.tensor_mul(ot[:], o_ps[:], g_bc[:])
                nc.sync.dma_start(moe_out[bass.ts(co, 128), csl], ot[:])

        # VE weighting for own token strip (token-major)
        for tt in range(4):
            rsl = bass.ts(tt, 128)
            r0 = stream.tile([128, C], f32, tag="ver0", name="ver0")
            r1 = stream.tile([128, C], f32, tag="ver1", name="ver1")
            nc.sync.dma_start(r0[:], ve0[rsl, :])
            nc.sync.dma_start(r1[:], ve1[rsl, :])
            nc.vector.tensor_scalar_mul(r0[:], r0[:], veg[:, 2 * tt:2 * tt + 1])
            nc.vector.tensor_scalar_mul(r1[:], r1[:],
                                        veg[:, 2 * tt + 1:2 * tt + 2])
            nc.vector.tensor_add(r0[:], r0[:], r1[:])
            nc.sync.dma_start(ve_out[rsl, :], r0[:])

    nc.compile()
    return nc


# --------------------------------------------------------------------------
# Host orchestration
# --------------------------------------------------------------------------
def _phase1_inputs(x, cos, sin, window, wq, wk, wv, wo, router_w):
    """Build per-core in_maps for phase 1."""
    cosT = np.ascontiguousarray(cos[0, :, 0, :].T)  # (32, T)
    sinT = np.ascontiguousarray(sin[0, :, 0, :].T)
    cosR = np.tile(cosT, (4, 1)).astype(np.float32)          # (128, T)
    sinR = np.tile(np.vstack([sinT, -sinT]), (2, 1)).astype(np.float32)

    # q-head placement permutation (see attention loop): head h lives at
    # chunk 2*(h//4)+(h%2), partition base 64*((h//2)%2)
    colmap = np.zeros(NH * HD, np.int64)
    for h in range(NH):
        pos = (2 * (h // 4) + (h % 2)) * 128 + 64 * ((h // 2) % 2)
        colmap[pos:pos + HD] = np.arange(h * HD, (h + 1) * HD)
    wqT = np.ascontiguousarray(wq.T[:, colmap])
    wkT = np.ascontiguousarray(wk.T)
    wvT = np.ascontiguousarray(wv.T)
    woT = np.ascontiguousarray(wo.T[colmap, :])
    rwT = np.ascontiguousarray(router_w.T)

    in_maps = []
    perms = []
    for c in range(NCORES):
        b, qi = c // 4, c % 4
        qoff = S * qi
        q128 = qoff // 128
        perm = [(q128 + s) % NSLOT for s in range(NSLOT)]
        perms.append(perm)
        xT = x[b].T  # (C, T)
        xT_rot = np.ascontiguousarray(
            xT.reshape(C, NSLOT, 128)[:, perm, :].reshape(C, T))
        cosR_b = np.ascontiguousarray(
            cosR.reshape(128, NSLOT, 128)[:, perm, :].reshape(128, T))
        sinR_b = np.ascontiguousarray(
            sinR.reshape(128, NSLOT, 128)[:, perm, :].reshape(128, T))
        # per-slot alive bias
        kbias = np.zeros((128, NSLOT), np.float32)
        for s in range(NSLOT):
            kt = perm[s]
            # any (q in [qoff, qoff+511], k in [kt*128, kt*128+127]) with
            # k <= q and q - k <= window?
            dmin = qoff - (kt * 128 + 127)
            dmax = qoff + S - 1 - kt * 128
            alive = (dmax >= 0) and (dmin <= window)
            if not alive:
                kbias[:, s] = -30000.0
        in_maps.append(dict(
            xT_b=xT_rot,
            cosR_b=cosR_b, sinR_b=sinR_b, kbias=kbias,
            wqT=wqT, wkT=wkT, wvT=wvT, woT=woT, rwT=rwT,
        ))
    return in_maps, perms


def _route(logits, router_bias):
    """Top-2 routing exactly as the reference (on host, f32)."""
    sig = (1.0 / (1.0 + np.exp(-logits.astype(np.float32)))).astype(np.float32)
    sel = sig + router_bias[None, :].astype(np.float32)
    idx = np.argsort(-sel, axis=1, kind="stable")[:, :TOPK]
    tw = np.take_along_axis(sig, idx, axis=1)
    tw = tw / tw.sum(axis=1, keepdims=True)
    N = logits.shape[0]
    sparse_w = np.zeros((N, E_MLP + E_VE), np.float32)
    np.put_along_axis(sparse_w, idx, tw, axis=1)
    return sparse_w


def kernel(**inputs):
    x = np.asarray(inputs["x"], np.float32)
    token_ids = np.asarray(inputs["token_ids"])
    cos = np.asarray(inputs["cos"], np.float32)
    sin = np.asarray(inputs["sin"], np.float32)
    window = int(np.asarray(inputs["window_size"]))
    wq, wk, wv, wo = (np.asarray(inputs[k], np.float32)
                      for k in ("wq", "wk", "wv", "wo"))
    w_up = np.asarray(inputs["w_up"], np.float32)
    w_down = np.asarray(inputs["w_down"], np.float32)
    router_w = np.asarray(inputs["router_w"], np.float32)
    router_bias = np.asarray(inputs["router_bias"], np.float32)
    ve_tables = np.asarray(inputs["ve_tables"], np.float32)

    key1 = ("p1", window)
    if key1 not in _prog_cache:
        _prog_cache[key1] = build_phase1(window)
    nc1 = _prog_cache[key1]

    in_maps, _ = _phase1_inputs(x, cos, sin, window, wq, wk, wv, wo, router_w)
    res1 = run_bass_kernel_spmd(nc1, in_maps, list(range(NCORES))).results

    x2T = np.concatenate([res1[c]["x2_out"] for c in range(NCORES)], axis=1)
    xfT = np.concatenate([res1[c]["xf_out"] for c in range(NCORES)], axis=1)
    logits = np.concatenate([res1[c]["logit_out"].T for c in range(NCORES)],
                            axis=0)  # (N, 10)

    N = B * T
    sparse_w = _route(logits, router_bias)

    # dispatch
    ncap = NCAP
    idx_list, n_list = [], []
    for e in range(E_MLP):
        idx_e = np.nonzero(sparse_w[:, e])[0]
        idx_list.append(idx_e)
        n_list.append(len(idx_e))
    max_n = max(n_list)
    while ncap < max_n:
        ncap *= 2

    key2 = ("p2", ncap)
    if key2 not in _prog_cache:
        _prog_cache[key2] = build_phase2(ncap)
    nc2 = _prog_cache[key2]

    tok = token_ids.reshape(-1)
    in_maps2 = []
    for c in range(NCORES):
        e = c
        idx_e = idx_list[e]
        xfg = np.zeros((C, ncap), np.float32)
        xfg[:, :n_list[e]] = xfT[:, idx_e]
        gate = np.zeros((1, ncap), np.float32)
        gate[0, :n_list[e]] = sparse_w[idx_e, e]
        s0 = c * S
        strip_tok = tok[s0:s0 + S]
        ve0 = np.ascontiguousarray(ve_tables[0][strip_tok])
        ve1 = np.ascontiguousarray(ve_tables[1][strip_tok])
        veg = np.zeros((128, 8), np.float32)
        for tt in range(4):
            for ee in range(E_VE):
                veg[:, 2 * tt + ee] = sparse_w[s0 + tt * 128:s0 + (tt + 1) * 128,
                                               E_MLP + ee]
        in_maps2.append(dict(
            xfg=xfg, w_upT=np.ascontiguousarray(w_up[e].T),
            w_downT=np.ascontiguousarray(w_down[e].T),
            gate=gate, ve0=ve0, ve1=ve1, ve_g=veg,
        ))
    res2 = run_bass_kernel_spmd(nc2, in_maps2, list(range(NCORES))).results

    out = np.ascontiguousarray(x2T.T)  # (N, C)
    for c in range(NCORES):
        out[c * S:(c + 1) * S] += res2[c]["ve_out"]
    for e in range(E_MLP):
        n_e = n_list[e]
        if n_e:
            out[idx_list[e]] += res2[e]["moe_out"][:, :n_e].T
    return out.reshape(B, T, C).astype(np.float32)


# revision 32
# speedup vs baseline: 13179.3690x; 1.0107x over previous
"""TRN2 Bass kernel for nn_BlockMoVaE (attention + MoE/VE routing block).

Self-contained: accepts FULL inputs, shards across 8 NeuronCores, returns
FULL output.

Sharding:
  Phase 1 (attention + router logits): token-parallel. Core c handles the
    512-query strip [qoff, qoff+512) of batch b=c//4, qoff=512*(c%4).
    Activations are kept FEATURE-major ([feature, token]) so no on-device
    transposes are needed. K/V are computed for the whole batch on each
    core of the batch group, with key 128-tiles stored in a per-core
    ROTATED slot order (slot s holds absolute key tile (qoff/128+s)%16) so
    the causal boundary lands at static slots 0..3 in every core's
    (shared, SPMD) program; fully-masked future tiles are killed by a
    per-slot additive bias (-3e4) inside the exp activation.
  Phase 2 (expert-parallel sparse MoE): core e computes MLP expert e over
    only the tokens routed to it (host gathers columns, capacity-padded);
    VE (vocab-embedding expert) rows are host-gathered and weighted on
    device per token strip. Host does top-2 routing between launches and
    the final scatter-add/assembly.

Matmuls run as float32r (full PE rate, ~1e-4 rel err); PSUM accumulates
in fp32.
"""
import numpy as np

import concourse.bass as bass
import concourse.bacc as bacc
import concourse.mybir as mybir
import concourse.tile as tile
from concourse.bass_utils import run_bass_kernel_spmd

# ---- problem constants (hardcoded per contest rules) ----
B, T, C = 2, 2048, 1024
NH, NKV, HD = 16, 8, 64
E_MLP, E_VE, TOPK = 8, 2, 2
HID = 2048
VOCAB = 50257
EPS = 1e-6
NCORES = 8
S = 512            # tokens per core strip
NSLOT = T // 128   # 16 key tiles per batch
NG = 4             # kv column groups of 512
NCAP = 1024        # expert token capacity (phase 2)

f32 = mybir.dt.float32
f32r = mybir.dt.float32r
bf16 = mybir.dt.bfloat16
AF = mybir.ActivationFunctionType

_prog_cache = {}



def _register_consts(nc, values):
    for value in values:
        key = (f32, float(value))
        if key not in nc.const_aps.aps:
            t = nc.alloc_sbuf_tensor(f"constap-{value}", [128, 1], f32)
            nc.gpsimd.memset(t.ap(), float(value))
            nc.const_aps.aps[key] = t.ap()
    nc.all_engine_barrier()


# --------------------------------------------------------------------------
# Phase 1 builder: attention + residual + rmsnorm + router logits
# --------------------------------------------------------------------------
def build_phase1(window: int):
    nc = bacc.Bacc("TRN2", target_bir_lowering=False, debug=False,
                   num_devices=NCORES)

    xT_b = nc.dram_tensor("xT_b", [C, T], f32r, kind="ExternalInput").ap()
    xT_s = xT_b[:, 0:S]          # strip == rotated slots 0..3
    cosR_b = nc.dram_tensor("cosR_b", [128, T], f32, kind="ExternalInput").ap()
    sinR_b = nc.dram_tensor("sinR_b", [128, T], f32, kind="ExternalInput").ap()
    cosR_s = cosR_b[:, 0:S]
    sinR_s = sinR_b[:, 0:S]
    kbias = nc.dram_tensor("kbias", [128, NSLOT], f32, kind="ExternalInput").ap()
    wqT = nc.dram_tensor("wqT", [C, NH * HD], f32r, kind="ExternalInput").ap()
    wkT = nc.dram_tensor("wkT", [C, NKV * HD], f32r, kind="ExternalInput").ap()
    wvT = nc.dram_tensor("wvT", [C, NKV * HD], f32r, kind="ExternalInput").ap()
    woT = nc.dram_tensor("woT", [C, C], f32r, kind="ExternalInput").ap()
    rwT = nc.dram_tensor("rwT", [C, E_MLP + E_VE], f32, kind="ExternalInput").ap()

    x2_out = nc.dram_tensor("x2_out", [C, S], f32, kind="ExternalOutput").ap()
    xf_out = nc.dram_tensor("xf_out", [C, S], f32r, kind="ExternalOutput").ap()
    logit_out = nc.dram_tensor("logit_out", [E_MLP + E_VE, S], f32,
                               kind="ExternalOutput").ap()

    _register_consts(nc, [EPS])
    from contextlib import ExitStack
    with tile.TileContext(nc) as tc, ExitStack() as est:
        const = est.enter_context(tc.tile_pool(name="const", bufs=1))
        ropes = est.enter_context(tc.tile_pool(name="ropes", bufs=1))
        ropeb = est.enter_context(tc.tile_pool(name="ropeb", bufs=1))
        wstream = est.enter_context(tc.tile_pool(name="wstream", bufs=2))
        wvp = est.enter_context(tc.tile_pool(name="wvp", bufs=1))
        xin = est.enter_context(tc.tile_pool(name="xin", bufs=1))
        kvp = est.enter_context(tc.tile_pool(name="kv", bufs=1))
        qp = est.enter_context(tc.tile_pool(name="qp", bufs=1))
        work = est.enter_context(tc.tile_pool(name="work", bufs=2))
        rows = est.enter_context(tc.tile_pool(name="rows", bufs=2))
        pexp = est.enter_context(tc.tile_pool(name="pexp", bufs=2))
        ypool = est.enter_context(tc.tile_pool(name="ypool", bufs=1))
        x2p = est.enter_context(tc.tile_pool(name="x2p", bufs=2))
        ps_acc = est.enter_context(tc.tile_pool(name="ps_acc", bufs=2, space="PSUM"))
        ps_row = est.enter_context(tc.tile_pool(name="ps_row", bufs=1, space="PSUM"))
        ps_bc = est.enter_context(tc.tile_pool(name="ps_bc", bufs=1, space="PSUM"))
        ps_att = est.enter_context(tc.tile_pool(name="ps_att", bufs=2, space="PSUM"))

        # ---- constants ----
        ones_col_f = const.tile([128, 1], f32, name="ones_col_f")
        nc.vector.memset(ones_col_f[:], 1.0)
        ones_col = const.tile([128, 1], f32r, name="ones_col")
        nc.scalar.copy(ones_col[:], ones_col_f[:])
        ones_row_f = const.tile([1, 128], f32, name="ones_row_f")
        nc.vector.memset(ones_row_f[:], 1.0)
        ones_row = const.tile([1, 128], f32r, name="ones_row")
        nc.scalar.copy(ones_row[:], ones_row_f[:])
        onescols = const.tile([128, NKV, 1], f32, name="onescols")
        nc.vector.memset(onescols[:], 1.0)
        onescols_r = const.tile([128, NKV, 1], f32r, name="onescols_r")
        nc.vector.tensor_copy(onescols_r[:], onescols[:])
        kb = const.tile([128, NSLOT], f32, name="kb")
        nc.sync.dma_start(kb[:], kbias[:])

        cs = ropes.tile([128, S], f32, name="cs")
        nc.sync.dma_start(cs[:], cosR_s[:])
        ss = ropes.tile([128, S], f32, name="ss")
        nc.sync.dma_start(ss[:], sinR_s[:])

        rw_t = [const.tile([128, E_MLP + E_VE], f32, tag=f"rw{i}",
                           name=f"rw{i}") for i in range(8)]
        wv_t = [wvp.tile([128, NKV * HD], f32r, tag=f"wv{i}", name=f"wv{i}")
                for i in range(8)]
        for i in range(8):
            nc.sync.dma_start(rw_t[i][:], rwT[bass.ts(i, 128), :])
            nc.sync.dma_start(wv_t[i][:], wvT[bass.ts(i, 128), :])

        # ---- helper: rms broadcast for feature-major tiles ----
        def rms_stats(xtiles, n, nfeat):
            ssq = ps_row.tile([1, n], f32, tag="row", name="ssq")
            for i, xt in enumerate(xtiles):
                sq = work.tile([128, n], f32r, tag="sqstat", name="sqstat")
                nc.vector.tensor_mul(sq[:], xt[:], xt[:])
                nc.tensor.matmul(ssq[:], ones_col[:], sq[:],
                                 start=(i == 0), stop=(i == len(xtiles) - 1))
            srow = rows.tile([1, n], f32, tag="srow", name="srow")
            nc.scalar.activation(srow[:], ssq[:], AF.Sqrt,
                                 bias=EPS, scale=1.0 / nfeat)
            rrow = rows.tile([1, n], f32r, tag="rrow", name="rrow")
            with nc.allow_low_precision(reason="f32r rms bcast rows"):
                nc.vector.reciprocal(rrow[:], srow[:])
            bc = ps_bc.tile([128, n], f32, tag="bc", name="bc")
            nc.tensor.matmul(bc[:], ones_row[:], rrow[:], start=True, stop=True)
            bcs = work.tile([128, n], f32, tag="bcstat", name="bcstat", bufs=1)
            nc.scalar.copy(bcs[:], bc[:])
            return bcs

        # ---- helper: rope + per-head rmsnorm on a projection psum ----
        def rope_norm(ps, cos_ap, sin_ap, n, out_tile, col0):
            # swp[blk] = ps[swapped 32-block] * sinR[blk] (psum offset reads)
            swp = work.tile([128, n], f32, tag="swp", name="swp")
            for blk in range(4):
                sb0 = (blk // 2) * 64 + (1 - blk % 2) * 32
                b0 = blk * 32
                nc.vector.tensor_mul(swp[b0:b0 + 32, :], ps[sb0:sb0 + 32, :],
                                     sin_ap[b0:b0 + 32, :])
            t1 = work.tile([128, n], f32, tag="ropet1", name="ropet1")
            nc.vector.tensor_mul(t1[:], ps[:], cos_ap)
            nc.vector.tensor_add(swp[:], t1[:], swp[:])   # roped value
            sq = work.tile([128, n], f32r, tag="ropet1", name="ropesq")
            nc.vector.tensor_mul(sq[:], swp[:], swp[:])
            for hh in range(2):
                p0 = 64 * hh
                ssqh = ps_row.tile([1, n], f32, tag="row", name="ssqh")
                nc.tensor.matmul(ssqh[:], ones_col[p0:p0 + 64, :],
                                 sq[p0:p0 + 64, :], start=True, stop=True)
                srow = rows.tile([1, n], f32, tag="srow", name="hsrow")
                nc.scalar.activation(srow[:], ssqh[:], AF.Sqrt,
                                     bias=EPS, scale=1.0 / HD)
                rrow = rows.tile([1, n], f32r, tag="rrow", name="hrrow")
                with nc.allow_low_precision(reason="f32r rms bcast rows"):
                    nc.vector.reciprocal(rrow[:], srow[:])
                bch = ps_bc.tile([64, n], f32, tag="bc", name="bch")
                nc.tensor.matmul(bch[:], ones_row[:, :64], rrow[:],
                                 start=True, stop=True)
                bcs = work.tile([128, n], f32, tag="hbc", name="hbc")
                nc.scalar.copy(bcs[p0:p0 + 64, :], bch[:])
                nc.vector.tensor_mul(
                    out_tile[p0:p0 + 64, col0:col0 + n],
                    swp[p0:p0 + 64, :], bcs[p0:p0 + 64, :])

        # ================= strip pipeline (Q) =================
        xs_t = [xin.tile([128, S], f32r, tag=f"xi{i}", name=f"xs{i}")
                for i in range(8)]
        for i in range(8):
            nc.sync.dma_start(xs_t[i][:], xT_s[bass.ts(i, 128), :])
        bc_s = rms_stats([t[:].bitcast(f32) for t in xs_t], S, C)
        xn_s = []
        for i in range(8):
            xr = xs_t[i][:]
            nc.vector.tensor_mul(xr, xr.bitcast(f32), bc_s[:])  # in-place norm
            xn_s.append(xr)

        qT = [qp.tile([128, S], f32r, tag=f"qT{i}", name=f"qT{i}")
              for i in range(8)]
        for dq in range(8):
            q_ps = ps_acc.tile([128, S], f32, tag="acc", name="q_ps")
            wsl = wstream.tile([128, C], f32r, tag="wq", name="wq_sl", bufs=1)
            nc.sync.dma_start(
                wsl[:].rearrange("p (a m) -> p a m", m=128),
                wqT[:, bass.ts(dq, 128)].rearrange("(a p) m -> p a m", p=128))
            for ci in range(8):
                nc.tensor.matmul(q_ps[:], wsl[:, bass.ts(ci, 128)], xn_s[ci],
                                 start=(ci == 0), stop=(ci == 7))
            rope_norm(q_ps, cs[:], ss[:], S, qT[dq], 0)

        # ================= batch pipeline (K, V) =================
        kT = [kvp.tile([128, T], f32r, tag=f"kT{i}", name=f"kT{i}")
              for i in range(4)]
        vaug = [kvp.tile([128, NKV * (HD + 1)], f32r, tag=f"va{i}",
                         name=f"va{i}") for i in range(NSLOT)]
        for g in range(NG):
            xb_t = [xin.tile([128, S], f32r, tag=f"xi{i}", name=f"xb{i}")
                    for i in range(8)]
            for i in range(8):
                nc.sync.dma_start(xb_t[i][:], xT_b[bass.ts(i, 128),
                                                   bass.ts(g, S)])
            cbg = ropeb.tile([128, S], f32, tag="cbg", name="cbg")
            nc.sync.dma_start(cbg[:], cosR_b[:, bass.ts(g, S)])
            sbg = ropeb.tile([128, S], f32, tag="sbg", name="sbg")
            nc.sync.dma_start(sbg[:], sinR_b[:, bass.ts(g, S)])
            bc_b = rms_stats([t[:].bitcast(f32) for t in xb_t], S, C)
            xn_b = []
            for i in range(8):
                xr = xb_t[i][:]
                nc.vector.tensor_mul(xr, xr.bitcast(f32), bc_b[:])
                xn_b.append(xr)
            for dk in range(4):
                k_ps = ps_acc.tile([128, S], f32, tag="acc", name="k_ps")
                wsl = wstream.tile([128, C], f32r, tag="wk", name="wk_sl",
                                   bufs=2)
                nc.sync.dma_start(
                    wsl[:].rearrange("p (a m) -> p a m", m=128),
                    wkT[:, bass.ts(dk, 128)].rearrange("(a p) m -> p a m",
                                                       p=128))
                for ci in range(8):
                    nc.tensor.matmul(k_ps[:], wsl[:, bass.ts(ci, 128)],
                                     xn_b[ci], start=(ci == 0), stop=(ci == 7))
                rope_norm(k_ps, cbg[:], sbg[:], S, kT[dk], g * S)
            for tt in range(4):
                slot = g * 4 + tt
                v_ps = ps_acc.tile([128, NKV * HD], f32, tag="acc", name="v_ps")
                for ci in range(8):
                    nc.tensor.matmul(v_ps[:],
                                     xn_b[ci][:, bass.ts(tt, 128)],
                                     wv_t[ci][:], start=(ci == 0), stop=(ci == 7))
                va = vaug[slot]
                va3 = va[:].rearrange("p (h d) -> p h d", d=HD + 1)
                vp3 = v_ps[:].rearrange("p (h d) -> p h d", d=HD)
                nc.scalar.copy(va3[:, :, 0:HD], vp3[:, :, :])
                nc.vector.tensor_copy(va3[:, :, HD:HD + 1], onescols_r[:])

        # ================= attention =================
        yT = [ypool.tile([128, S], f32r, tag=f"yT{i}", name=f"yT{i}")
              for i in range(8)]
        for h in range(NH):
            kh = h // 2                       # kv head
            dk, kp0 = kh // 2, 64 * (kh % 2)  # kT chunk/partition offset
            # q head layout is host-permuted so its partition base matches
            # the kv head base (matmul requires equal bases)
            dq, qp0 = 2 * (h // 4) + (h % 2), 64 * ((h // 2) % 2)
            assert qp0 == kp0
            yv = ps_att.tile([HD + 1, S], f32, tag="yv", name="yv", bufs=2)
            for sp in range(NSLOT // 2):
                # two slots share one 2-bank psum tile and one exp op; the
                # per-slot dead bias is pair-uniform (dead range is slots
                # 4..15-qoff/128, always whole pairs)
                s2 = ps_acc.tile([128, 2 * S], f32, tag="acc", name="s2")
                for half in range(2):
                    s = 2 * sp + half
                    nc.tensor.matmul(
                        s2[:, half * S:(half + 1) * S],
                        kT[dk][kp0:kp0 + 64, bass.ts(s, 128)],
                        qT[dq][qp0:qp0 + 64, :], start=True, stop=True)
                pT = pexp.tile([128, 2 * S], f32r, tag="pT", name="pT")
                nc.scalar.activation(pT[:], s2[:], AF.Exp,
                                     bias=kb[:, 2 * sp:2 * sp + 1], scale=0.125)
                for half in range(2):
                    s = 2 * sp + half
                    pTh = pT[:, half * S:(half + 1) * S]
                    if s < 4:
                        nc.gpsimd.affine_select(
                            pTh, pTh, pattern=[[1, S]], base=-128 * s,
                            channel_multiplier=-1,
                            compare_op=mybir.AluOpType.is_ge, fill=0.0)
                        if window < 512 - 128 * s:
                            nc.gpsimd.affine_select(
                                pTh, pTh, pattern=[[1, S]],
                                base=-128 * s - window, channel_multiplier=-1,
                                compare_op=mybir.AluOpType.is_le, fill=0.0)
                    else:
                        # cover partially AND fully window-cut past slots:
                        # a fully-cut slot may be pair-unmasked (kbias is
                        # pair-granular), so affine-zero it here
                        m = NSLOT - s
                        if window < 128 * m + 511:
                            nc.gpsimd.affine_select(
                                pTh, pTh, pattern=[[1, S]],
                                base=128 * m - window, channel_multiplier=-1,
                                compare_op=mybir.AluOpType.is_le, fill=0.0)
                    nc.tensor.matmul(yv[:], vaug[s][:, 65 * kh:65 * kh + 65],
                                     pTh, start=(s == 0), stop=(s == NSLOT - 1))
            ry = rows.tile([1, S], f32r, tag="ry", name="ry")
            with nc.allow_low_precision(reason="f32r softmax denom row"):
                nc.vector.reciprocal(ry[:], yv[HD:HD + 1, :])
            bc_y = ps_bc.tile([64, S], f32, tag="bc", name="bc_y")
            nc.tensor.matmul(bc_y[:], ones_row[:, :64], ry[:],
                             start=True, stop=True)
            bcy_s = work.tile([128, S], f32, tag="hbc", name="bcy")
            nc.vector.tensor_copy(bcy_s[qp0:qp0 + 64, :], bc_y[:])
            nc.vector.tensor_mul(yT[dq][qp0:qp0 + 64, :], yv[0:HD, :],
                                 bcy_s[qp0:qp0 + 64, :])

        # ================= wo + residual + xf + router =================
        x2w = []
        for co in range(8):
            at_ps = ps_acc.tile([128, S], f32, tag="acc", name="at_ps")
            wsl = wstream.tile([128, C], f32r, tag="wo", name="wo_sl", bufs=2)
            nc.sync.dma_start(
                wsl[:].rearrange("p (a m) -> p a m", m=128),
                woT[:, bass.ts(co, 128)].rearrange("(a p) m -> p a m", p=128))
            for ci in range(8):
                nc.tensor.matmul(at_ps[:], wsl[:, bass.ts(ci, 128)],
                                 yT[ci][:], start=(ci == 0), stop=(ci == 7))
            xs2 = xin.tile([128, S], f32r, tag=f"xi{co}", name=f"xs2_{co}")
            nc.sync.dma_start(xs2[:], xT_s[bass.ts(co, 128), :])
            x2 = x2p.tile([128, S], f32, tag="x2w", name="x2w")
            nc.vector.tensor_add(x2[:], at_ps[:], xs2[:].bitcast(f32))
            nc.sync.dma_start(x2_out[bass.ts(co, 128), :], x2[:])
            x2w.append(x2)
        # xf stats need all 8 chunks: re-read x2 from DRAM to save SBUF
        x2r = [xin.tile([128, S], f32, tag=f"xi{i}", name=f"x2r{i}")
               for i in range(8)]
        for i in range(8):
            nc.sync.dma_start(x2r[i][:], x2_out[bass.ts(i, 128), :])
        bc_f = rms_stats(x2r, S, C)
        rt_ps = ps_row.tile([E_MLP + E_VE, S], f32, tag="row", name="rt_ps")
        for i in range(8):
            xf = x2p.tile([128, S], f32r, tag="xf", name="xf")
            nc.vector.tensor_mul(xf[:], x2r[i][:], bc_f[:])
            nc.sync.dma_start(xf_out[bass.ts(i, 128), :], xf[:])
            nc.tensor.matmul(rt_ps[:], rw_t[i][:], xf[:].bitcast(f32),
                             start=(i == 0), stop=(i == 7))
        lg = rows.tile([E_MLP + E_VE, S], f32, tag="lg", name="lg", bufs=1)
        nc.scalar.copy(lg[:], rt_ps[:])
        nc.sync.dma_start(logit_out[:], lg[:])

    nc.compile()
    return nc


# --------------------------------------------------------------------------
# Phase 2 builder: sparse expert MLP + VE weighting
# --------------------------------------------------------------------------
def build_phase2(ncap: int):
    nc = bacc.Bacc("TRN2", target_bir_lowering=False, debug=False,
                   num_devices=NCORES)
    NT = ncap // 256

    xfg = nc.dram_tensor("xfg", [C, ncap], f32r, kind="ExternalInput").ap()
    w_upT = nc.dram_tensor("w_upT", [C, HID], f32r, kind="ExternalInput").ap()
    w_downT = nc.dram_tensor("w_downT", [HID, C], f32r,
                             kind="ExternalInput").ap()
    gate = nc.dram_tensor("gate", [1, ncap], f32, kind="ExternalInput").ap()
    ve0 = nc.dram_tensor("ve0", [S, C], f32, kind="ExternalInput").ap()
    ve1 = nc.dram_tensor("ve1", [S, C], f32, kind="ExternalInput").ap()
    ve_g = nc.dram_tensor("ve_g", [128, 8], f32, kind="ExternalInput").ap()

    moe_out = nc.dram_tensor("moe_out", [C, ncap], f32, kind="ExternalOutput").ap()
    ve_out = nc.dram_tensor("ve_out", [S, C], f32, kind="ExternalOutput").ap()

    from contextlib import ExitStack
    with tile.TileContext(nc) as tc, ExitStack() as est:
        const = est.enter_context(tc.tile_pool(name="const", bufs=1))
        wpool = est.enter_context(tc.tile_pool(name="wpool", bufs=1))
        hpool = est.enter_context(tc.tile_pool(name="hpool", bufs=1))
        stream = est.enter_context(tc.tile_pool(name="stream", bufs=2))
        work = est.enter_context(tc.tile_pool(name="work", bufs=2))
        ps_h = est.enter_context(tc.tile_pool(name="ps_h", bufs=2, space="PSUM"))
        ps_o = est.enter_context(tc.tile_pool(name="ps_o", bufs=2, space="PSUM"))
        ps_b = est.enter_context(tc.tile_pool(name="ps_b", bufs=2, space="PSUM"))

        ones_row = const.tile([1, 128], f32)
        nc.vector.memset(ones_row[:], 1.0)

        up_t = [wpool.tile([128, HID], f32r, tag=f"up{i}", name=f"up{i}") for i in range(8)]
        dn_t = [wpool.tile([128, C], f32r, tag=f"dn{i}", name=f"dn{i}") for i in range(16)]
        for i in range(8):
            nc.sync.dma_start(up_t[i][:], w_upT[bass.ts(i, 128), :])
        for i in range(16):
            nc.sync.dma_start(dn_t[i][:], w_downT[bass.ts(i, 128), :])
        veg = const.tile([128, 8], f32)
        nc.sync.dma_start(veg[:], ve_g[:])
        gate_sb = const.tile([1, ncap], f32)
        nc.sync.dma_start(gate_sb[:], gate[:])

        for nt in range(NT):
            csl = bass.ts(nt, 256)
            xf_t = [stream.tile([128, 256], f32r, tag=f"xf{i}", name=f"xf{i}")
                    for i in range(8)]
            for i in range(8):
                nc.sync.dma_start(xf_t[i][:], xfg[bass.ts(i, 128), csl])
            g_ps = ps_b.tile([128, 256], f32)
            nc.tensor.matmul(g_ps[:], ones_row[:], gate_sb[:, csl],
                             start=True, stop=True)
            g_bc = work.tile([128, 256], f32, tag="gbc", name="gbc")
            nc.scalar.copy(g_bc[:], g_ps[:])
            hT = [hpool.tile([128, 256], f32r, tag=f"hT{i}", name=f"hT{i}")
                  for i in range(16)]
            for hc in range(16):
                h_ps = ps_h.tile([128, 256], f32)
                for ci in range(8):
                    nc.tensor.matmul(h_ps[:], up_t[ci][:, bass.ts(hc, 128)],
                                     xf_t[ci][:], start=(ci == 0),
                                     stop=(ci == 7))
                hr = work.tile([128, 256], f32, tag="hrelu", name="hrelu")
                nc.scalar.activation(hr[:], h_ps[:], AF.Relu)
                nc.vector.tensor_mul(hT[hc][:], hr[:], hr[:])
            for co in range(8):
                o_ps = ps_o.tile([128, 256], f32)
                for hc in range(16):
                    nc.tensor.matmul(o_ps[:], dn_t[hc][:, bass.ts(co, 128)],
                                     hT[hc][:], start=(hc == 0),
                                     stop=(hc == 15))
                ot = work.tile([128, 256], f32, tag="ot", name="ot")
                nc.vector.tensor_mul(ot[:], o_ps[:], g_bc[:])
                nc.sync.dma_start(moe_out[bass.ts(co, 128), csl], ot[:])

        # VE weighting for own token strip (token-major)
        for tt in range(4):
            rsl = bass.ts(tt, 128)
            r0 = stream.tile([128, C], f32, tag="ver0", name="ver0")
            r1 = stream.tile([128, C], f32, tag="ver1", name="ver1")
            nc.sync.dma_start(r0[:], ve0[rsl, :])
            nc.sync.dma_start(r1[:], ve1[rsl, :])
            nc.vector.tensor_scalar_mul(r0[:], r0[:], veg[:, 2 * tt:2 * tt + 1])
            nc.vector.tensor_scalar_mul(r1[:], r1[:],
                                        veg[:, 2 * tt + 1:2 * tt + 2])
            nc.vector.tensor_add(r0[:], r0[:], r1[:])
            nc.sync.dma_start(ve_out[rsl, :], r0[:])

    nc.compile()
    return nc


# --------------------------------------------------------------------------
# Host orchestration
# --------------------------------------------------------------------------
def _phase1_inputs(x, cos, sin, window, wq, wk, wv, wo, router_w):
    """Build per-core in_maps for phase 1."""
    cosT = np.ascontiguousarray(cos[0, :, 0, :].T)  # (32, T)
    sinT = np.ascontiguousarray(sin[0, :, 0, :].T)
    cosR = np.tile(cosT, (4, 1)).astype(np.float32)          # (128, T)
    sinR = np.tile(np.vstack([sinT, -sinT]), (2, 1)).astype(np.float32)

    # q-head placement permutation (see attention loop): head h lives at
    # chunk 2*(h//4)+(h%2), partition base 64*((h//2)%2)
    colmap = np.zeros(NH * HD, np.int64)
    for h in range(NH):
        pos = (2 * (h // 4) + (h % 2)) * 128 + 64 * ((h // 2) % 2)
        colmap[pos:pos + HD] = np.arange(h * HD, (h + 1) * HD)
    wqT = np.ascontiguousarray(wq.T[:, colmap])
    wkT = np.ascontiguousarray(wk.T)
    wvT = np.ascontiguousarray(wv.T)
    woT = np.ascontiguousarray(wo.T[colmap, :])
    rwT = np.ascontiguousarray(router_w.T)

    in_maps = []
    perms = []
    for c in range(NCORES):
        b, qi = c // 4, c % 4
        qoff = S * qi
        q128 = qoff // 128
        perm = [(q128 + s) % NSLOT for s in range(NSLOT)]
        perms.append(perm)
        xT = x[b].T  # (C, T)
        xT_rot = np.ascontiguousarray(
            xT.reshape(C, NSLOT, 128)[:, perm, :].reshape(C, T))
        cosR_b = np.ascontiguousarray(
            cosR.reshape(128, NSLOT, 128)[:, perm, :].reshape(128, T))
        sinR_b = np.ascontiguousarray(
            sinR.reshape(128, NSLOT, 128)[:, perm, :].reshape(128, T))
        # per-slot alive bias
        kbias = np.zeros((128, NSLOT), np.float32)
        alive_s = np.zeros(NSLOT, bool)
        for s in range(NSLOT):
            kt = perm[s]
            # any (q in [qoff, qoff+511], k in [kt*128, kt*128+127]) with
            # k <= q and q - k <= window?
            dmin = qoff - (kt * 128 + 127)
            dmax = qoff + S - 1 - kt * 128
            alive_s[s] = (dmax >= 0) and (dmin <= window)
        for sp in range(NSLOT // 2):
            # the device applies one bias per slot PAIR; window-cut dead
            # slots in a live pair are zeroed by the device affine instead
            if not (alive_s[2 * sp] or alive_s[2 * sp + 1]):
                kbias[:, 2 * sp:2 * sp + 2] = -30000.0
        in_maps.append(dict(
            xT_b=xT_rot,
            cosR_b=cosR_b, sinR_b=sinR_b, kbias=kbias,
            wqT=wqT, wkT=wkT, wvT=wvT, woT=woT, rwT=rwT,
        ))
    return in_maps, perms


def _route(logits, router_bias):
    """Top-2 routing exactly as the reference (on host, f32)."""
    sig = (1.0 / (1.0 + np.exp(-logits.astype(np.float32)))).astype(np.float32)
    sel = sig + router_bias[None, :].astype(np.float32)
    idx = np.argsort(-sel, axis=1, kind="stable")[:, :TOPK]
    tw = np.take_along_axis(sig, idx, axis=1)
    tw = tw / tw.sum(axis=1, keepdims=True)
    N = logits.shape[0]
    sparse_w = np.zeros((N, E_MLP + E_VE), np.float32)
    np.put_along_axis(sparse_w, idx, tw, axis=1)
    return sparse_w


def kernel(**inputs):
    x = np.asarray(inputs["x"], np.float32)
    token_ids = np.asarray(inputs["token_ids"])
    cos = np.asarray(inputs["cos"], np.float32)
    sin = np.asarray(inputs["sin"], np.float32)
    window = int(np.asarray(inputs["window_size"]))
    wq, wk, wv, wo = (np.asarray(inputs[k], np.float32)
                      for k in ("wq", "wk", "wv", "wo"))
    w_up = np.asarray(inputs["w_up"], np.float32)
    w_down = np.asarray(inputs["w_down"], np.float32)
    router_w = np.asarray(inputs["router_w"], np.float32)
    router_bias = np.asarray(inputs["router_bias"], np.float32)
    ve_tables = np.asarray(inputs["ve_tables"], np.float32)

    key1 = ("p1", window)
    if key1 not in _prog_cache:
        _prog_cache[key1] = build_phase1(window)
    nc1 = _prog_cache[key1]

    in_maps, _ = _phase1_inputs(x, cos, sin, window, wq, wk, wv, wo, router_w)
    res1 = run_bass_kernel_spmd(nc1, in_maps, list(range(NCORES))).results

    x2T = np.concatenate([res1[c]["x2_out"] for c in range(NCORES)], axis=1)
    xfT = np.concatenate([res1[c]["xf_out"] for c in range(NCORES)], axis=1)
    logits = np.concatenate([res1[c]["logit_out"].T for c in range(NCORES)],
                            axis=0)  # (N, 10)

    N = B * T
    sparse_w = _route(logits, router_bias)

    # dispatch
    ncap = NCAP
    idx_list, n_list = [], []
    for e in range(E_MLP):
        idx_e = np.nonzero(sparse_w[:, e])[0]
        idx_list.append(idx_e)
        n_list.append(len(idx_e))
    max_n = max(n_list)
    while ncap < max_n:
        ncap *= 2

    key2 = ("p2", ncap)
    if key2 not in _prog_cache:
        _prog_cache[key2] = build_phase2(ncap)
    nc2 = _prog_cache[key2]

    tok = token_ids.reshape(-1)
    in_maps2 = []
    for c in range(NCORES):
        e = c
        idx_e = idx_list[e]
        xfg = np.zeros((C, ncap), np.float32)
        xfg[:, :n_list[e]] = xfT[:, idx_e]
        gate = np.zeros((1, ncap), np.float32)
        gate[0, :n_list[e]] = sparse_w[idx_e, e]
        s0 = c * S
        strip_tok = tok[s0:s0 + S]
        ve0 = np.ascontiguousarray(ve_tables[0][strip_tok])
        ve1 = np.ascontiguousarray(ve_tables[1][strip_tok])
        veg = np.zeros((128, 8), np.float32)
        for tt in range(4):
            for ee in range(E_VE):
                veg[:, 2 * tt + ee] = sparse_w[s0 + tt * 128:s0 + (tt + 1) * 128,
                                               E_MLP + ee]
        in_maps2.append(dict(
            xfg=xfg, w_upT=np.ascontiguousarray(w_up[e].T),
            w_downT=np.ascontiguousarray(w_down[e].T),
            gate=gate, ve0=ve0, ve1=ve1, ve_g=veg,
        ))
    res2 = run_bass_kernel_spmd(nc2, in_maps2, list(range(NCORES))).results

    out = np.ascontiguousarray(x2T.T)  # (N, C)
    for c in range(NCORES):
        out[c * S:(c + 1) * S] += res2[c]["ve_out"]
    for e in range(E_MLP):
        n_e = n_list[e]
        if n_e:
            out[idx_list[e]] += res2[e]["moe_out"][:, :n_e].T
    return out.reshape(B, T, C).astype(np.float32)
